# revision 2
# baseline (speedup 1.0000x reference)
"""ButterflyGatingUnit Trainium2 kernel, v2.

Contract: kernel(**inputs) takes FULL inputs (x: [8, 96, 128, 128] + weights/
scalars), returns FULL output [8, 96, 128, 128] f32. Data-parallel over batch,
one example per NeuronCore, one SPMD Bass program.

v2 structure (vs baseline):
 - Phase A: ONE stats pass over x (stats of relu(x) at threshold 0 +
   analytic correction to threshold m) instead of two; sign-form emit pass
   split across Act/DVE/Pool engines, all bf16.
 - val/cv/a stay in SBUF (no DRAM round trips); y2t precomputed into x2pad's
   space; xr round-trips DRAM in bf16.
 - B1 transposes batched 5-per-PSUM-tile before eviction; evictions
   alternate Act/DVE.
"""
import numpy as np
import ml_dtypes
import concourse.bass as bass
import concourse.tile as tile
import concourse.mybir as mybir
from concourse.masks import make_identity
from concourse.bass_utils import run_bass_kernel_spmd
from contextlib import ExitStack

# ---------------------------------------------------------------------------
# Patch TileContext._drain_and_barrier: this walrus build rejects Drain
# instructions carrying more than one sem wait. Split the final global-clock
# wait set across a chain of Drain instructions on SP, one wait each.
from concourse.vector_clock import ScopedClock

MAX_WAITS_PER_DRAIN = 1


def _patched_drain_and_barrier(self, tick_clock, wait_clock):
    nc = self.nc
    drain_inst = nc.sync.drain()
    wait_clock.add_sem_waits(
        drain_inst.ins, ScopedClock({None: tick_clock.global_clock})
    )
    inst = drain_inst.ins
    si = inst.sync_info
    waits = list(si.on_wait) if (si and si.on_wait) else []
    if len(waits) > MAX_WAITS_PER_DRAIN:
        si.on_wait = waits[:MAX_WAITS_PER_DRAIN]
        rest = waits[MAX_WAITS_PER_DRAIN:]
        while rest:
            extra = nc.sync.drain()
            extra.ins.sync_info = mybir.SyncInfo(
                on_wait=rest[:MAX_WAITS_PER_DRAIN], on_update=[]
            )
            rest = rest[MAX_WAITS_PER_DRAIN:]

    nc.all_engine_barrier()
    assert self.sems is not None
    popped = nc._tile_sem_poison_stack.pop()
    assert popped is self._sem_poison
    nc.clear_and_free_semaphores(list(self.sems.allocated().values()))
    nc.all_engine_barrier()


tile.TileContext._drain_and_barrier = _patched_drain_and_barrier
# ---------------------------------------------------------------------------

AF = mybir.ActivationFunctionType
ALU = mybir.AluOpType
F32 = mybir.dt.float32
BF16 = mybir.dt.bfloat16

C = 96
KK = 9

MAX_WAITS_PER_INST = 1


def _split_multi_waits(nc):
    """This walrus build encodes at most one sem wait per instruction. Hoist
    extra waits onto NoOp carriers inserted just before, on the same engine."""
    f = nc.m.functions[0]
    for blk in f.blocks:
        insts = blk.instructions
        new = []
        changed = False
        ctr = 0
        for inst in insts:
            si = inst.sync_info
            waits = list(si.on_wait) if (si and si.on_wait) else []
            if len(waits) > MAX_WAITS_PER_INST:
                changed = True
                while len(waits) > MAX_WAITS_PER_INST:
                    chunk = waits[:MAX_WAITS_PER_INST]
                    waits = waits[MAX_WAITS_PER_INST:]
                    nop = mybir.InstNoOp(
                        name=f"{inst.name}-wsplit{ctr}", engine=inst.engine,
                        ins=[], outs=[],
                        sync_info=mybir.SyncInfo(on_wait=chunk, on_update=[]))
                    try:
                        nc.register_instruction(nop, overwrite=True)
                    except Exception:
                        pass
                    new.append(nop)
                    ctr += 1
                si.on_wait = waits
            new.append(inst)
        if changed:
            blk.instructions = new


def sub_ap(t_ap, row0, col0, nrow, ncol, rstep, cstep):
    """Strided 3D view [C, nrow, ncol] of a padded [C, PH, PW] SBUF tile."""
    base = t_ap[:, row0, col0]
    pstep = t_ap.ap[0][0]
    row_elems = t_ap.ap[-2][0]
    return bass.AP(
        tensor=base.tensor,
        offset=base.offset,
        ap=[[pstep, C], [row_elems * rstep, nrow], [cstep, ncol]],
    )


def flat_ap(t_ap, start, n):
    """1D slice [C, n] at elem offset `start` of any [C, ...] SBUF tile."""
    pstep = t_ap.ap[0][0]
    return bass.AP(tensor=t_ap.tensor, offset=t_ap.offset + start,
                   ap=[[pstep, C], [1, n]])


def build_kernel(nc, H, W, sc1, sc2, res_coef, nc1v, nc2v):
    HW = H * W
    total = float(C * HW)
    Ho = (H - 3) // 3 + 1
    Wo = (W - 3) // 3 + 1
    L = Ho * Wo
    PH, PW = H + 2, W + 4      # interior origin: row 1, col 2
    NT = H // 4                # 4-row blocks
    assert H % 4 == 0 and W % 4 == 0
    eps = 1e-5

    CW = 2048                  # phase-A chunk width (elems per partition)
    NCH = HW // CW             # 8 chunks
    RPC = CW // W              # rows per chunk (16)

    plc = max(1, min(126 // Wo, Ho))                 # ph rows per transpose blk
    npc = plc * max(1, min(504 // (plc * Wo), (Ho + plc - 1) // plc))
    lcnt_max = plc * Wo                              # 126

    # ---------------- DRAM ----------------
    x_in = nc.dram_tensor("x", [C, H * W], F32, kind="ExternalInput").ap()
    aw1t = nc.dram_tensor("aw1t", [C, KK * C], BF16, kind="ExternalInput").ap()
    aw2t = nc.dram_tensor("aw2t", [C, KK * C], BF16, kind="ExternalInput").ap()
    aw3t = nc.dram_tensor("aw3t", [C, KK * C], BF16, kind="ExternalInput").ap()
    wc2t = nc.dram_tensor("wc2t", [C, KK * C], BF16, kind="ExternalInput").ap()
    wf1 = nc.dram_tensor("wf1", [C, C], F32, kind="ExternalInput").ap()
    wf2 = nc.dram_tensor("wf2", [C, C], BF16, kind="ExternalInput").ap()
    bfull = nc.dram_tensor("bfull", [C, 1], F32, kind="ExternalInput").ap()
    out_d = nc.dram_tensor("out", [C, H * W], F32, kind="ExternalOutput").ap()

    with tile.TileContext(nc) as tc, ExitStack() as ctx:
        dram = ctx.enter_context(tc.tile_pool(name="dram", bufs=1, space="DRAM"))
        xr_d = dram.tile([C, H * W], BF16)

        # ---------------- big SBUF tiles ----------------
        big = ctx.enter_context(tc.tile_pool(name="big", bufs=1))
        x1pad = big.tile([C, PH, PW], BF16)
        x2pad = big.tile([C, PH, PW], BF16)   # later carved: y2t = flat 16384
        valp = big.tile([C, PH, PW], BF16)    # value conv output (padded)
        cv_sb = big.tile([C, HW], BF16)       # cv; earlier: pass-3 work arena
        a_sb = big.tile([C, HW], BF16)        # attn-conv out; earlier: u0 ring

        wpool = ctx.enter_context(tc.tile_pool(name="wpool", bufs=1))
        aw1_sb = wpool.tile([C, KK, C], BF16)
        aw2_sb = wpool.tile([C, KK, C], BF16)
        aw3_sb = wpool.tile([C, KK, C], BF16)
        wc2_sb = wpool.tile([C, KK, C], BF16)
        wf1_sb = wpool.tile([C, C], F32)
        wf1s_sb = wpool.tile([C, C], BF16)
        wf2_sb = wpool.tile([C, C], BF16)
        bfull_sb = wpool.tile([C, 1], F32)
        ident = wpool.tile([128, 128], BF16)
        identf = wpool.tile([128, 128], F32)
        ones_c = wpool.tile([C, 1], F32)
        ones_row = wpool.tile([1, C], F32)
        for dst, src in [(aw1_sb, aw1t), (aw2_sb, aw2t), (aw3_sb, aw3t),
                         (wc2_sb, wc2t), (wf2_sb, wf2)]:
            d = dst[:].rearrange("c a b -> c (a b)") if len(dst.shape) == 3 else dst
            nc.sync.dma_start(out=d, in_=src)
        nc.sync.dma_start(out=wf1_sb, in_=wf1)
        nc.sync.dma_start(out=bfull_sb, in_=bfull)
        make_identity(nc, ident)
        make_identity(nc, identf)
        nc.vector.memset(ones_c, 1.0)
        nc.vector.memset(ones_row, 1.0)

        # zero pad borders once (interiors rewritten below)
        for pad in (x1pad, x2pad, valp):
            nc.vector.memset(pad[:, 0, :], 0.0)
            nc.vector.memset(pad[:, PH - 1, :], 0.0)
            nc.vector.memset(pad[:, :, 0:2], 0.0)
            nc.vector.memset(pad[:, :, W + 2:W + 4], 0.0)

        # ---------------- stats / scalar pools ----------------
        st = ctx.enter_context(tc.tile_pool(name="st", bufs=1))
        bnstats = st.tile([C, 4 * NCH, 6], F32)
        mv = st.tile([C, 2], F32)
        pm = st.tile([C, 2], F32)
        s1part = st.tile([C, NCH], F32)
        q1part = st.tile([C, NCH], F32)
        pospart = st.tile([C, NCH], F32)
        p3 = st.tile([C, 3], F32)
        sc = st.tile([1, 48], F32)      # scalar scratch lane
        cstv = st.tile([1, 24], F32)    # consts to broadcast
        cst = st.tile([C, 24], F32)     # broadcast result
        astats = st.tile([C, NT, 6], F32)
        cvstats = st.tile([C, NT, 6], F32)
        amv = st.tile([C, 2], F32)
        cvmv = st.tile([C, 2], F32)
        lnp = st.tile([C, 4], F32)
        lns = st.tile([1, 8], F32)
        lnb = st.tile([C, 8], F32)
        corr = st.tile([C, 1], F32)
        attn_sb = st.tile([C, KK, C], F32)
        attnT_sb = st.tile([C, KK, C], BF16)
        mx = st.tile([C, 1], F32)
        negmx = st.tile([C, 1], F32)
        den = st.tile([C, 1], F32)
        rden = st.tile([C, 1], F32)
        cs1 = st.tile([C, 1], F32)
        kct = st.tile([1, 8], F32)
        nc.vector.memset(kct[:, 0:1], total)
        nc.vector.memset(kct[:, 1:2], eps)
        nc.vector.memset(kct[:, 2:3], float(sc1))
        nc.vector.memset(kct[:, 3:4], float(sc2))
        nc.vector.memset(kct[:, 4:5], float(res_coef))
        K_TOTAL = kct[:, 0:1]; K_EPS = kct[:, 1:2]
        K_SC1 = kct[:, 2:3]; K_SC2 = kct[:, 3:4]; K_RES = kct[:, 4:5]

        # ---------------- PSUM pools ----------------
        psC = ctx.enter_context(tc.tile_pool(name="psC", bufs=3, space="PSUM"))
        psT = ctx.enter_context(tc.tile_pool(name="psT", bufs=2, space="PSUM"))
        psA = ctx.enter_context(tc.tile_pool(name="psA", bufs=1, space="PSUM"))
        psS = ctx.enter_context(tc.tile_pool(name="psS", bufs=1, space="PSUM"))

        # ---------------- small rings ----------------
        xt_pool = ctx.enter_context(tc.tile_pool(name="xt", bufs=2))
        ev_pool = ctx.enter_context(tc.tile_pool(name="ev", bufs=2))
        kq_pool = ctx.enter_context(tc.tile_pool(name="kq", bufs=3))
        io_pool = ctx.enter_context(tc.tile_pool(name="io", bufs=2))

        def bcast(dst_ck, src_1k, k):
            """broadcast [1,k] -> [C,k] via ones matmul."""
            ps = psS.tile([C, 32], F32, tag="small")
            nc.tensor.matmul(out=ps[:, :k], lhsT=ones_row, rhs=src_1k,
                             start=True, stop=True)
            nc.vector.tensor_copy(dst_ck, ps[:, :k])

        xr_flat = xr_d[:]
        x2flat = flat_ap(x2pad[:], 0, HW)   # y2t arena (x2pad reused)

        # =========================================================
        # Phase A1: single stats pass over x
        #   bn_stats(x) -> per-channel mean/ex2;  u0 = relu(x) (Act, accum S1)
        #   Square(u0) (Act, accum Q1);  is_gt (DVE, accum POS)
        # =========================================================
        for c in range(NCH):
            xt = xt_pool.tile([C, CW], F32, tag="xt")
            nc.sync.dma_start(out=xt, in_=x_in[:, c * CW:(c + 1) * CW])
            xtv = xt.rearrange("c (a b) -> c a b", a=4)
            for j in range(4):
                nc.vector.bn_stats(out=bnstats[:, 4 * c + j, :], in_=xtv[:, j, :])
            u0 = flat_ap(a_sb[:], (c % 2) * CW, CW)
            nc.scalar.activation(out=u0, in_=xt, func=AF.Relu,
                                 accum_out=s1part[:, c:c + 1])
            sq = flat_ap(a_sb[:], (2 + (c % 2)) * CW, CW)
            nc.scalar.activation(out=sq, in_=u0, func=AF.Square,
                                 accum_out=q1part[:, c:c + 1])
            gt = flat_ap(a_sb[:], (4 + (c % 2)) * CW, CW)
            nc.vector.tensor_scalar(out=gt, in0=u0, scalar1=0.0, scalar2=None,
                                    op0=ALU.is_gt, op1=ALU.add,
                                    accum_out=pospart[:, c:c + 1])

        # =========================================================
        # Soup: aggregate + threshold-correct + branch consts
        # =========================================================
        nc.vector.bn_aggr(out=mv, in_=bnstats)
        # pm = [mean_c, ex2_c]
        nc.vector.tensor_tensor(out=pm[:, 0:1], in0=mv[:, 0:1], in1=mv[:, 0:1],
                                op=ALU.mult)
        nc.vector.tensor_tensor(out=pm[:, 1:2], in0=mv[:, 1:2], in1=pm[:, 0:1],
                                op=ALU.add)
        nc.vector.tensor_copy(pm[:, 0:1], mv[:, 0:1])
        # per-channel partial reductions for S1/Q1/POS at threshold 0
        nc.vector.reduce_sum(out=p3[:, 0:1], in_=s1part, axis=mybir.AxisListType.X)
        nc.vector.reduce_sum(out=p3[:, 1:2], in_=pospart, axis=mybir.AxisListType.X)
        nc.vector.reduce_sum(out=p3[:, 2:3], in_=q1part, axis=mybir.AxisListType.X)
        psm = psS.tile([C, 32], F32, tag="small")
        nc.tensor.matmul(out=psm[:1, 0:2], lhsT=ones_c, rhs=pm, start=True, stop=True)
        ps3 = psS.tile([C, 32], F32, tag="small")
        nc.tensor.matmul(out=ps3[:1, 0:3], lhsT=ones_c, rhs=p3, start=True, stop=True)
        Smean = sc[:, 0:1]; Sex2 = sc[:, 1:2]
        nc.vector.tensor_copy(Smean, psm[:1, 0:1])
        nc.vector.tensor_copy(Sex2, psm[:1, 1:2])
        S10 = sc[:, 2:3]; POS = sc[:, 3:4]; Q10 = sc[:, 4:5]
        nc.vector.tensor_copy(S10, ps3[:1, 0:1])
        nc.vector.tensor_copy(POS, ps3[:1, 1:2])
        nc.vector.tensor_copy(Q10, ps3[:1, 2:3])
        m_ = sc[:, 5:6]
        nc.scalar.mul(out=m_, in_=Smean, mul=1.0 / C)
        Sx2 = sc[:, 6:7]
        nc.scalar.mul(out=Sx2, in_=Sex2, mul=float(HW))
        mm_ = sc[:, 7:8]
        nc.vector.tensor_tensor(out=mm_, in0=m_, in1=m_, op=ALU.mult)
        qtot = sc[:, 8:9]
        nc.vector.scalar_tensor_tensor(out=qtot, in0=mm_, scalar=-total, in1=Sx2,
                                       op0=ALU.mult, op1=ALU.add)
        # threshold corrections: S1 = S10 - m*POS ; Q1 = Q10 - 2m*S10 + m^2*POS
        t0 = sc[:, 9:10]; t1s = sc[:, 10:11]
        S1 = sc[:, 11:12]; Q1 = sc[:, 12:13]
        nc.vector.tensor_tensor(out=t0, in0=m_, in1=POS, op=ALU.mult)
        nc.vector.tensor_tensor(out=S1, in0=S10, in1=t0, op=ALU.subtract)
        nc.vector.tensor_tensor(out=t0, in0=m_, in1=S10, op=ALU.mult)
        nc.vector.scalar_tensor_tensor(out=t0, in0=t0, scalar=-2.0, in1=Q10,
                                       op0=ALU.mult, op1=ALU.add)
        nc.vector.tensor_tensor(out=t1s, in0=mm_, in1=POS, op=ALU.mult)
        nc.vector.tensor_tensor(out=Q1, in0=t0, in1=t1s, op=ALU.add)

        NEG = sc[:, 13:14]
        # NEG = total - POS  (activation trick needs [C,1] bias; do via lane)
        nc.scalar.mul(out=NEG, in_=POS, mul=-1.0)
        nc.vector.tensor_scalar(out=NEG, in0=NEG, scalar1=total, scalar2=None,
                                op0=ALU.add)
        rPOS = sc[:, 14:15]; rNEG = sc[:, 15:16]
        nc.vector.reciprocal(out=rPOS, in_=POS)
        nc.vector.reciprocal(out=rNEG, in_=NEG)
        avg1 = sc[:, 16:17]
        nc.vector.tensor_tensor(out=avg1, in0=S1, in1=rPOS, op=ALU.mult)
        nS1 = sc[:, 17:18]
        nc.scalar.mul(out=nS1, in_=S1, mul=-1.0)
        avg2 = sc[:, 18:19]
        nc.vector.tensor_tensor(out=avg2, in0=nS1, in1=rNEG, op=ALU.mult)
        q2 = sc[:, 19:20]
        nc.vector.tensor_tensor(out=q2, in0=qtot, in1=Q1, op=ALU.subtract)

        def ln_branch(Ssum, Qsum, avg, CNT_other, CNT_own, o_mean, o_scale, tmp0):
            ta = sc[:, tmp0:tmp0 + 1]
            tb = sc[:, tmp0 + 1:tmp0 + 2]
            nc.vector.tensor_tensor(out=ta, in0=avg, in1=CNT_other, op=ALU.mult)
            nc.vector.tensor_tensor(out=tb, in0=Ssum, in1=ta, op=ALU.add)
            nc.scalar.mul(out=o_mean, in_=tb, mul=1.0 / total)
            nc.vector.tensor_tensor(out=ta, in0=avg, in1=avg, op=ALU.mult)
            nc.vector.tensor_tensor(out=ta, in0=ta, in1=CNT_other, op=ALU.mult)
            nc.vector.tensor_tensor(out=ta, in0=Qsum, in1=ta, op=ALU.add)
            nc.scalar.mul(out=ta, in_=ta, mul=1.0 / total)
            nc.vector.tensor_tensor(out=tb, in0=o_mean, in1=o_mean, op=ALU.mult)
            nc.vector.tensor_tensor(out=ta, in0=ta, in1=tb, op=ALU.subtract)
            nc.scalar.activation(out=ta, in_=ta, func=AF.Sqrt, bias=K_EPS, scale=1.0)
            nc.vector.reciprocal(out=ta, in_=ta)
            nc.scalar.mul(out=tb, in_=CNT_own, mul=1.0 / total)
            nc.scalar.activation(out=tb, in_=tb, func=AF.Sqrt, bias=0.0, scale=1.0)
            nc.vector.tensor_tensor(out=o_scale, in0=tb, in1=ta, op=ALU.mult)

        mean1 = sc[:, 20:21]; scale1 = sc[:, 21:22]
        mean2 = sc[:, 22:23]; scale2 = sc[:, 23:24]
        ln_branch(S1, Q1, avg1, NEG, POS, mean1, scale1, 24)
        ln_branch(nS1, q2, avg2, POS, NEG, mean2, scale2, 24)

        a1 = sc[:, 26:27]; b1 = sc[:, 27:28]; c1n = sc[:, 28:29]
        a2 = sc[:, 29:30]; b2 = sc[:, 30:31]; c2n = sc[:, 31:32]
        nc.scalar.activation(out=a1, in_=scale1, func=AF.Identity, bias=K_SC1, scale=1.0)
        nc.vector.tensor_tensor(out=t0, in0=scale1, in1=mean1, op=ALU.mult)
        nc.scalar.mul(out=b1, in_=t0, mul=-1.0)
        nc.vector.tensor_tensor(out=t0, in0=avg1, in1=mean1, op=ALU.subtract)
        nc.vector.tensor_tensor(out=c1n, in0=scale1, in1=t0, op=ALU.mult)
        nc.scalar.activation(out=a2, in_=scale2, func=AF.Identity, bias=K_SC2, scale=1.0)
        nc.vector.tensor_tensor(out=t0, in0=scale2, in1=mean2, op=ALU.mult)
        nc.scalar.mul(out=b2, in_=t0, mul=-1.0)
        nc.vector.tensor_tensor(out=t0, in0=avg2, in1=mean2, op=ALU.subtract)
        nc.vector.tensor_tensor(out=c2n, in0=scale2, in1=t0, op=ALU.mult)
        p1 = sc[:, 32:33]; p2 = sc[:, 33:34]
        nc.scalar.activation(out=p1, in_=scale1, func=AF.Identity,
                             bias=K_RES, scale=0.5 * nc1v)
        nc.scalar.activation(out=p2, in_=scale2, func=AF.Identity,
                             bias=K_RES, scale=0.5 * nc2v)
        q1c = sc[:, 34:35]; q2c = sc[:, 35:36]
        nc.scalar.mul(out=t0, in_=b1, mul=0.5 * nc1v)
        nc.vector.scalar_tensor_tensor(out=q1c, in0=c2n, scalar=0.5 * nc2v, in1=t0,
                                       op0=ALU.mult, op1=ALU.add)
        nc.scalar.mul(out=t0, in_=c1n, mul=0.5 * nc1v)
        nc.vector.scalar_tensor_tensor(out=q2c, in0=b2, scalar=0.5 * nc2v, in1=t0,
                                       op0=ALU.mult, op1=ALU.add)

        # sign-form consts (s = sign(x-m) in {-1,+1}):
        #  x1' = a1*u + B1*s + G1,  B1 = (b1-c1n)/2, G1 = (b1+c1n)/2
        #  x2' = -(a2*un) + B2*s + G2, B2 = (c2n-b2)/2, G2 = (c2n+b2)/2
        #  xr  = (p1/a1)*(a1 u) - (p2/a2)*(a2 un) + B3*s + G3,
        #        B3 = (q1c-q2c)/2, G3 = (q1c+q2c)/2
        # cstv cols: 0:-m 1:a1 2:-a1*m 3:-a2 4:a2*m 5:B1 6:G1 7:B2 8:G2
        #            9:B3 10:G3 11:p1/a1 12:-p2/a2
        nc.scalar.mul(out=cstv[:, 0:1], in_=m_, mul=-1.0)
        nc.vector.tensor_copy(cstv[:, 1:2], a1)
        nc.vector.tensor_tensor(out=cstv[:, 2:3], in0=a1, in1=cstv[:, 0:1],
                                op=ALU.mult)
        nc.scalar.mul(out=cstv[:, 3:4], in_=a2, mul=-1.0)
        nc.vector.tensor_tensor(out=cstv[:, 4:5], in0=a2, in1=m_, op=ALU.mult)
        # p-form: w1 = (b1-c1n)*p + c1n ; w2 = (c2n-b2)*p + b2 ;
        #         wr = (q1c-q2c)*p + q2c   with p = (xm>0) in {0,1}
        nc.vector.tensor_tensor(out=cstv[:, 5:6], in0=b1, in1=c1n, op=ALU.subtract)
        nc.vector.tensor_copy(cstv[:, 6:7], c1n)
        nc.vector.tensor_tensor(out=cstv[:, 7:8], in0=c2n, in1=b2, op=ALU.subtract)
        nc.vector.tensor_copy(cstv[:, 8:9], b2)
        nc.vector.tensor_tensor(out=cstv[:, 9:10], in0=q1c, in1=q2c, op=ALU.subtract)
        nc.vector.tensor_copy(cstv[:, 10:11], q2c)
        nc.vector.reciprocal(out=t0, in_=a1)
        nc.vector.tensor_tensor(out=cstv[:, 11:12], in0=p1, in1=t0, op=ALU.mult)
        nc.vector.reciprocal(out=t0, in_=a2)
        nc.vector.tensor_tensor(out=t1s, in0=p2, in1=t0, op=ALU.mult)
        nc.scalar.mul(out=cstv[:, 12:13], in_=t1s, mul=-1.0)
        nc.scalar.mul(out=cstv[:, 13:14], in_=p2, mul=-1.0)
        nc.vector.tensor_tensor(out=cstv[:, 14:15], in0=p2, in1=m_, op=ALU.mult)
        bcast(cst[:, 0:15], cstv[:, 0:15], 15)
        NEGM = cst[:, 0:1]; A1C = cst[:, 1:2]; A1NM = cst[:, 2:3]
        NA2C = cst[:, 3:4]; A2M = cst[:, 4:5]
        B1C = cst[:, 5:6]; G1C = cst[:, 6:7]
        B2C = cst[:, 7:8]; G2C = cst[:, 8:9]
        B3C = cst[:, 9:10]; G3C = cst[:, 10:11]
        P1R = cst[:, 11:12]; NP2R = cst[:, 12:13]
        NP2C = cst[:, 13:14]; P2M = cst[:, 14:15]

        # =========================================================
        # Pass 3: emit x1', x2' (padded tiles) + xr (DRAM, bf16)
        # work slots carved from cv_sb: 8 slots x 2 parity x CW
        # =========================================================
        def slot(k, par):
            base = cv_sb if k < 4 else a_sb
            kk_ = k if k < 4 else k - 4
            return bass.AP(tensor=base[:].tensor,
                           offset=base[:].offset + (kk_ * 2 + par) * CW,
                           ap=[[base[:].ap[0][0], C], [1, CW]])

        for c in range(NCH):
            par = c % 2
            xt = xt_pool.tile([C, CW], F32, tag="xt")
            nc.sync.dma_start(out=xt, in_=x_in[:, c * CW:(c + 1) * CW])
            s_t = slot(0, par); au_t = slot(1, par); aun_t = slot(2, par)
            aunr_t = slot(3, par)
            w1_t = slot(4, par); w2_t = slot(5, par); wr_t = slot(6, par)
            r1_t = slot(7, par); xr1_t = slot(4, par)  # reuses w1 slot
            nc.scalar.activation(out=au_t, in_=xt, func=AF.Relu, bias=A1NM, scale=A1C)
            nc.vector.tensor_scalar(out=s_t, in0=au_t, scalar1=0.0, scalar2=None,
                                    op0=ALU.is_gt)
            nc.scalar.activation(out=aun_t, in_=xt, func=AF.Relu, bias=A2M, scale=NA2C)
            nc.scalar.activation(out=aunr_t, in_=xt, func=AF.Relu, bias=P2M, scale=NP2C)
            x1v = sub_ap(x1pad[:], 1 + c * RPC, 2, RPC, W, 1, 1)
            x2v = sub_ap(x2pad[:], 1 + c * RPC, 2, RPC, W, 1, 1)
            nc.vector.tensor_scalar(out=w1_t, in0=s_t, scalar1=B1C, scalar2=G1C,
                                    op0=ALU.mult, op1=ALU.add)
            nc.vector.tensor_tensor(out=x1v, in0=au_t, in1=w1_t, op=ALU.add)
            nc.vector.tensor_scalar(out=w2_t, in0=s_t, scalar1=B2C, scalar2=G2C,
                                    op0=ALU.mult, op1=ALU.add)
            nc.vector.tensor_tensor(out=x2v, in0=w2_t, in1=aun_t, op=ALU.subtract)
            nc.vector.tensor_scalar(out=wr_t, in0=s_t, scalar1=B3C, scalar2=G3C,
                                    op0=ALU.mult, op1=ALU.add)
            nc.vector.tensor_scalar(out=r1_t, in0=au_t, scalar1=P1R, scalar2=None,
                                    op0=ALU.mult)
            nc.vector.tensor_tensor(out=xr1_t, in0=r1_t, in1=wr_t, op=ALU.add)
            # xr = xr1 - aun_r  (gpsimd lane), into r1's slot
            nc.gpsimd.tensor_tensor(out=r1_t, in0=xr1_t, in1=aunr_t,
                                    op=ALU.subtract)
            nc.sync.dma_start(out=xr_flat[:, c * CW:(c + 1) * CW], in_=r1_t)

        # =========================================================
        # B1: attention logits, per kk: strided K/Q convs + batched
        # transposes + attn matmul accumulation
        # =========================================================
        NSLOT = 4  # transposes batched per psT tile before eviction
        ev_ctr = 0
        for kk in range(KK):
            r, s = divmod(kk, 3)
            attn_ps = psA.tile([C, C], F32, tag="attn")
            first = True
            # accumulated list of (SBUF kq tile, col offset, lcnt)
            pend = []       # transposes in current psT not yet evicted
            done_slices = []  # (sb_tile, off, lcnt, is_q)
            tp_cur = None
            tp_used = 0

            def flush_tp():
                nonlocal tp_cur, tp_used, pend, ev_ctr
                if tp_cur is None or tp_used == 0:
                    return
                sb = kq_pool.tile([128, NSLOT * C], BF16, tag="kq")
                eng = nc.scalar if (ev_ctr % 2 == 0) else nc.vector
                ev_ctr += 1
                if eng is nc.scalar:
                    nc.scalar.activation(out=sb[:, :tp_used * C],
                                         in_=tp_cur[:, :tp_used * C], func=AF.Copy)
                else:
                    nc.vector.tensor_copy(sb[:, :tp_used * C],
                                          tp_cur[:, :tp_used * C])
                for (i, lcnt, is_q) in pend:
                    done_slices.append((sb, i * C, lcnt, is_q))
                tp_cur = None; tp_used = 0; pend = []

            def add_transpose(src_ap, lcnt, is_q):
                nonlocal tp_cur, tp_used, pend
                if tp_cur is None:
                    tp_cur = psT.tile([128, NSLOT * C], BF16, tag="tp")
                    tp_used = 0
                nc.tensor.transpose(tp_cur[:lcnt, tp_used * C:tp_used * C + C],
                                    src_ap, ident[:C, :C])
                pend.append((tp_used, lcnt, is_q))
                tp_used += 1
                if tp_used == NSLOT:
                    flush_tp()

            ph0 = 0
            while ph0 < Ho:
                this = min(npc, Ho - ph0)
                N = this * Wo
                kc_ps = psC.tile([C, 512], F32, tag="conv")
                qc_ps = psC.tile([C, 512], F32, tag="conv")
                for tap in range(KK):
                    dy, dx = divmod(tap, 3)
                    rhs1 = sub_ap(x1pad[:], 3 * ph0 + r + dy, 1 + s + dx, this, Wo, 3, 3)
                    rhs2 = sub_ap(x2pad[:], 3 * ph0 + r + dy, 1 + s + dx, this, Wo, 3, 3)
                    nc.tensor.matmul(out=kc_ps[:, :N], lhsT=aw1_sb[:, tap, :],
                                     rhs=rhs1, start=(tap == 0), stop=(tap == 8))
                    nc.tensor.matmul(out=qc_ps[:, :N], lhsT=aw2_sb[:, tap, :],
                                     rhs=rhs2, start=(tap == 0), stop=(tap == 8))
                kc = ev_pool.tile([C, 512], BF16, tag="kc")
                qc = ev_pool.tile([C, 512], BF16, tag="qc")
                nc.scalar.activation(out=kc[:, :N], in_=kc_ps[:, :N], func=AF.Copy)
                nc.vector.tensor_copy(qc[:, :N], qc_ps[:, :N])
                l0 = 0
                while l0 < N:
                    lcnt = min(lcnt_max, N - l0)
                    add_transpose(kc[:, l0:l0 + lcnt], lcnt, False)
                    add_transpose(qc[:, l0:l0 + lcnt], lcnt, True)
                    l0 += lcnt
                ph0 += this
            flush_tp()
            # attn matmuls: pair consecutive (k, q) slices in order
            ks = [d for d in done_slices if not d[3]]
            qs = [d for d in done_slices if d[3]]
            for i, ((ksb, koff, lcnt, _), (qsb, qoff, _, _)) in enumerate(zip(ks, qs)):
                nc.tensor.matmul(out=attn_ps,
                                 lhsT=qsb[:lcnt, qoff:qoff + C],
                                 rhs=ksb[:lcnt, koff:koff + C],
                                 start=(i == 0), stop=(i == len(ks) - 1))
            nc.scalar.activation(out=attn_sb[:, kk, :], in_=attn_ps, func=AF.Copy)

        # ---------------- softmax over (kk, c) ----------------
        nc.vector.reduce_max(out=mx, in_=attn_sb, axis=mybir.AxisListType.XY)
        nc.scalar.mul(out=negmx, in_=mx, mul=-1.0)
        nc.scalar.activation(out=attn_sb, in_=attn_sb, func=AF.Exp, bias=negmx,
                             scale=1.0, accum_out=den)
        nc.vector.reciprocal(out=rden, in_=den)

        # =========================================================
        # B3: Cv conv (x2pad) -> cv_sb + stats
        # =========================================================
        for yb in range(NT):
            pt = psC.tile([C, 512], F32, tag="conv")
            for tap in range(KK):
                dy, dx = divmod(tap, 3)
                rhs = sub_ap(x2pad[:], yb * 4 + dy, 1 + dx, 4, W, 1, 1)
                nc.tensor.matmul(out=pt[:, :4 * W], lhsT=wc2_sb[:, tap, :], rhs=rhs,
                                 start=(tap == 0), stop=(tap == 8))
            cv_v = flat_ap(cv_sb[:], yb * 4 * W, 4 * W)
            if yb % 2 == 0:
                nc.scalar.activation(out=cv_v, in_=pt[:, :4 * W], func=AF.Copy)
            else:
                nc.vector.tensor_copy(cv_v, pt[:, :4 * W])
            nc.vector.bn_stats(out=cvstats[:, yb, :], in_=cv_v)

        # cv LN consts
        nc.vector.bn_aggr(out=cvmv, in_=cvstats)
        nc.vector.tensor_scalar(out=lnp[:, 2:3], in0=cvmv[:, 0:1],
                                scalar1=float(HW), scalar2=None, op0=ALU.mult)
        nc.vector.tensor_tensor(out=lnp[:, 3:4], in0=cvmv[:, 0:1],
                                in1=cvmv[:, 0:1], op=ALU.mult)
        nc.vector.tensor_tensor(out=lnp[:, 3:4], in0=lnp[:, 3:4],
                                in1=cvmv[:, 1:2], op=ALU.add)
        nc.vector.tensor_scalar(out=lnp[:, 3:4], in0=lnp[:, 3:4],
                                scalar1=float(HW), scalar2=None, op0=ALU.mult)
        psc2 = psS.tile([C, 32], F32, tag="small")
        nc.tensor.matmul(out=psc2[:1, 0:2], lhsT=ones_c, rhs=lnp[:, 2:4],
                         start=True, stop=True)
        sCv = lns[:, 2:3]; qCv = lns[:, 3:4]
        nc.vector.tensor_copy(sCv, psc2[:1, 0:1])
        nc.vector.tensor_copy(qCv, psc2[:1, 1:2])

        def ln_const(ssum, qsum, o_mean, o_rs, ta, tb):
            nc.scalar.mul(out=o_mean, in_=ssum, mul=1.0 / total)
            nc.scalar.mul(out=ta, in_=qsum, mul=1.0 / total)
            nc.vector.tensor_tensor(out=tb, in0=o_mean, in1=o_mean, op=ALU.mult)
            nc.vector.tensor_tensor(out=ta, in0=ta, in1=tb, op=ALU.subtract)
            nc.scalar.activation(out=ta, in_=ta, func=AF.Sqrt, bias=K_EPS, scale=1.0)
            nc.vector.reciprocal(out=o_rs, in_=ta)

        tmpa = sc[:, 40:41]; tmpb = sc[:, 41:42]
        mCv = lns[:, 6:7]; rsCv = lns[:, 7:8]
        ln_const(sCv, qCv, mCv, rsCv, tmpa, tmpb)
        nc.vector.tensor_copy(lns[:, 0:1], mCv)
        nc.vector.tensor_copy(lns[:, 1:2], rsCv)
        bcast(lnb[:, 0:2], lns[:, 0:2], 2)
        MCV = lnb[:, 0:1]; RSCV = lnb[:, 1:2]

        # =========================================================
        # B4: value conv (x1pad) -> valp (padded, SBUF)
        # =========================================================
        for yb in range(NT):
            pt = psC.tile([C, 512], F32, tag="conv")
            for tap in range(KK):
                dy, dx = divmod(tap, 3)
                rhs = sub_ap(x1pad[:], yb * 4 + dy, 1 + dx, 4, W, 1, 1)
                nc.tensor.matmul(out=pt[:, :4 * W], lhsT=aw3_sb[:, tap, :], rhs=rhs,
                                 start=(tap == 0), stop=(tap == 8))
            vv = sub_ap(valp[:], 1 + yb * 4, 2, 4, W, 1, 1)
            if yb % 2 == 0:
                nc.vector.tensor_copy(vv, pt[:, :4 * W].rearrange(
                    "c (a b) -> c a b", a=4))
            else:
                nc.scalar.activation(out=vv, in_=pt[:, :4 * W].rearrange(
                    "c (a b) -> c a b", a=4), func=AF.Copy)

        # =========================================================
        # y2t = ((cv - mCv) * rsCv) * x1  -> x2pad arena (bf16)
        # =========================================================
        for c in range(NCH):
            cv_v = flat_ap(cv_sb[:], c * CW, CW)
            y2a = bass.AP(tensor=a_sb[:].tensor,
                          offset=a_sb[:].offset + (c % 2) * CW,
                          ap=[[a_sb[:].ap[0][0], C], [W, RPC], [1, W]])  # scratch
            nc.vector.tensor_scalar(out=y2a,
                                    in0=cv_v.rearrange("c (a b) -> c a b", a=RPC),
                                    scalar1=MCV, scalar2=RSCV,
                                    op0=ALU.subtract, op1=ALU.mult)
            x1v = sub_ap(x1pad[:], 1 + c * RPC, 2, RPC, W, 1, 1)
            y2v = bass.AP(tensor=x2pad[:].tensor,
                          offset=x2pad[:].offset + c * CW,
                          ap=[[x2pad[:].ap[0][0], C], [W, RPC], [1, W]])
            nc.vector.tensor_tensor(out=y2v, in0=y2a, in1=x1v, op=ALU.mult)

        # =========================================================
        # B5: w_attn transposes (f32 -> bf16 attnT)
        # =========================================================
        for kk in range(KK):
            tp = psA.tile([128, C], F32, tag="tpf")
            nc.tensor.transpose(tp[:C, :], attn_sb[:, kk, :], identf[:C, :C])
            nc.scalar.activation(out=attnT_sb[:, kk, :], in_=tp[:C, :], func=AF.Copy)

        # prefetch xr (DRAM) into cv_sb arena for phase D
        for c in range(4):
            nc.sync.dma_start(out=flat_ap(cv_sb[:], c * (HW // 4), HW // 4),
                              in_=xr_flat[:, c * (HW // 4):(c + 1) * (HW // 4)])

        # =========================================================
        # B6: A conv (attnT @ valp) -> a_sb + stats (rden folded at evict)
        # =========================================================
        for yb in range(NT):
            pt = psC.tile([C, 512], F32, tag="conv")
            for tap in range(KK):
                dy, dx = divmod(tap, 3)
                rhs = sub_ap(valp[:], yb * 4 + dy, 1 + dx, 4, W, 1, 1)
                nc.tensor.matmul(out=pt[:, :4 * W], lhsT=attnT_sb[:, tap, :],
                                 rhs=rhs, start=(tap == 0), stop=(tap == 8))
            a_v = flat_ap(a_sb[:], yb * 4 * W, 4 * W)
            nc.scalar.activation(out=a_v, in_=pt[:, :4 * W], func=AF.Copy,
                                 scale=rden)
            nc.vector.bn_stats(out=astats[:, yb, :], in_=a_v)

        # ---------------- A LN consts -> wf1s, corr ----------------
        nc.vector.bn_aggr(out=amv, in_=astats)
        nc.vector.tensor_scalar(out=lnp[:, 0:1], in0=amv[:, 0:1],
                                scalar1=float(HW), scalar2=None, op0=ALU.mult)
        nc.vector.tensor_tensor(out=lnp[:, 1:2], in0=amv[:, 0:1],
                                in1=amv[:, 0:1], op=ALU.mult)
        nc.vector.tensor_tensor(out=lnp[:, 1:2], in0=lnp[:, 1:2],
                                in1=amv[:, 1:2], op=ALU.add)
        nc.vector.tensor_scalar(out=lnp[:, 1:2], in0=lnp[:, 1:2],
                                scalar1=float(HW), scalar2=None, op0=ALU.mult)
        ps4 = psS.tile([C, 32], F32, tag="small")
        nc.tensor.matmul(out=ps4[:1, 0:2], lhsT=ones_c, rhs=lnp[:, 0:2],
                         start=True, stop=True)
        sA = lns[:, 4:5]; qA = lns[:, 5:6]
        nc.vector.tensor_copy(sA, ps4[:1, 0:1])
        nc.vector.tensor_copy(qA, ps4[:1, 1:2])
        mA = lns[:, 6:7]; rsA = lns[:, 7:8]
        ln_const(sA, qA, mA, rsA, tmpa, tmpb)
        nc.vector.tensor_copy(lns[:, 2:3], rsA)
        nc.vector.tensor_copy(lns[:, 3:4], mA)
        bcast(lnb[:, 2:4], lns[:, 2:4], 2)
        RSA = lnb[:, 2:3]; MA_ = lnb[:, 3:4]
        # wf1s = wf1 * rsA (bf16)
        nc.vector.tensor_scalar_mul(out=wf1s_sb, in0=wf1_sb, scalar1=RSA)
        # corr = bfull - rsA*mA*colsum(wf1)
        psc = psS.tile([C, 32], F32, tag="small")
        nc.tensor.matmul(out=psc[:, 0:1], lhsT=wf1_sb, rhs=ones_c,
                         start=True, stop=True)
        nc.vector.tensor_copy(cs1, psc[:, 0:1])
        nc.vector.tensor_scalar_mul(out=cs1, in0=cs1, scalar1=RSA)
        nc.vector.tensor_scalar_mul(out=cs1, in0=cs1, scalar1=MA_)
        nc.vector.tensor_tensor(out=corr, in0=bfull_sb, in1=cs1, op=ALU.subtract)

        # =========================================================
        # Phase D: out = (wf1s@a + wf2@y2t) + corr + xr
        # =========================================================
        for yb in range(NT):
            a_v = flat_ap(a_sb[:], yb * 4 * W, 4 * W)
            y2_v = flat_ap(x2pad[:], yb * 4 * W, 4 * W)
            xr_v = flat_ap(cv_sb[:], yb * 4 * W, 4 * W)
            pt = psC.tile([C, 512], F32, tag="conv")
            nc.tensor.matmul(out=pt[:, :4 * W], lhsT=wf1s_sb, rhs=a_v,
                             start=True, stop=False)
            nc.tensor.matmul(out=pt[:, :4 * W], lhsT=wf2_sb, rhs=y2_v,
                             start=False, stop=True)
            ot = io_pool.tile([C, 4 * W], F32, tag="ot")
            nc.vector.scalar_tensor_tensor(out=ot, in0=pt[:, :4 * W], scalar=corr,
                                           in1=xr_v, op0=ALU.add, op1=ALU.add)
            nc.sync.dma_start(out=out_d[:, yb * 4 * W:(yb + 1) * 4 * W], in_=ot)

    _split_multi_waits(nc)
    return nc


_NC_CACHE = {}


def _get_nc(H, W, sc1, sc2, res_coef, nc1v, nc2v):
    key = (H, W, float(sc1), float(sc2), float(res_coef), float(nc1v), float(nc2v))
    if key not in _NC_CACHE:
        nc = bass.Bass("TRN2", target_bir_lowering=False, debug=False)
        build_kernel(nc, H, W, float(sc1), float(sc2), float(res_coef),
                     float(nc1v), float(nc2v))
        _NC_CACHE[key] = nc
    return _NC_CACHE[key]


def _prep_w(w, scale=1.0):
    return np.ascontiguousarray(
        (np.asarray(w, np.float32).transpose(1, 2, 3, 0).reshape(C, 9 * C) * scale)
    ).astype(ml_dtypes.bfloat16)


def kernel(x, w_conv2, aw1, aw2, aw3, w_full, b_full, sc1, sc2, res_coef, nc1, nc2):
    x = np.asarray(x, np.float32)
    B, Cc, H, W = x.shape
    assert Cc == C
    nc = _get_nc(H, W, sc1, sc2, res_coef, nc1, nc2)

    inv_s = 1.0 / np.sqrt(C * 9.0)
    w_full = np.asarray(w_full, np.float32)
    shared = {
        "aw1t": _prep_w(aw1, inv_s),
        "aw2t": _prep_w(aw2),
        "aw3t": _prep_w(aw3),
        "wc2t": _prep_w(w_conv2),
        "wf1": np.ascontiguousarray(w_full[:, :C, 0, 0].T).astype(np.float32),
        "wf2": np.ascontiguousarray(w_full[:, C:, 0, 0].T).astype(ml_dtypes.bfloat16),
        "bfull": np.asarray(b_full, np.float32).reshape(C, 1),
    }
    in_maps = [{"x": np.ascontiguousarray(x[b]).reshape(C, H * W), **shared}
               for b in range(B)]
    res = run_bass_kernel_spmd(nc, in_maps, core_ids=list(range(B)))
    return np.stack([res.results[b]["out"].reshape(C, H, W) for b in range(B)],
                    axis=0)


# revision 3
# speedup vs baseline: 1.0047x; 1.0047x over previous
"""ButterflyGatingUnit Trainium2 kernel, v2.

Contract: kernel(**inputs) takes FULL inputs (x: [8, 96, 128, 128] + weights/
scalars), returns FULL output [8, 96, 128, 128] f32. Data-parallel over batch,
one example per NeuronCore, one SPMD Bass program.

v2 structure (vs baseline):
 - Phase A: ONE stats pass over x (stats of relu(x) at threshold 0 +
   analytic correction to threshold m) instead of two; sign-form emit pass
   split across Act/DVE/Pool engines, all bf16.
 - val/cv/a stay in SBUF (no DRAM round trips); y2t precomputed into x2pad's
   space; xr round-trips DRAM in bf16.
 - B1 transposes batched 5-per-PSUM-tile before eviction; evictions
   alternate Act/DVE.
"""
import numpy as np
import ml_dtypes
import concourse.bass as bass
import concourse.tile as tile
import concourse.mybir as mybir
from concourse.masks import make_identity
from concourse.bass_utils import run_bass_kernel_spmd
from contextlib import ExitStack

# ---------------------------------------------------------------------------
# Patch TileContext._drain_and_barrier: this walrus build rejects Drain
# instructions carrying more than one sem wait. Split the final global-clock
# wait set across a chain of Drain instructions on SP, one wait each.
from concourse.vector_clock import ScopedClock

MAX_WAITS_PER_DRAIN = 1


def _patched_drain_and_barrier(self, tick_clock, wait_clock):
    nc = self.nc
    drain_inst = nc.sync.drain()
    wait_clock.add_sem_waits(
        drain_inst.ins, ScopedClock({None: tick_clock.global_clock})
    )
    inst = drain_inst.ins
    si = inst.sync_info
    waits = list(si.on_wait) if (si and si.on_wait) else []
    if len(waits) > MAX_WAITS_PER_DRAIN:
        si.on_wait = waits[:MAX_WAITS_PER_DRAIN]
        rest = waits[MAX_WAITS_PER_DRAIN:]
        while rest:
            extra = nc.sync.drain()
            extra.ins.sync_info = mybir.SyncInfo(
                on_wait=rest[:MAX_WAITS_PER_DRAIN], on_update=[]
            )
            rest = rest[MAX_WAITS_PER_DRAIN:]

    nc.all_engine_barrier()
    assert self.sems is not None
    popped = nc._tile_sem_poison_stack.pop()
    assert popped is self._sem_poison
    nc.clear_and_free_semaphores(list(self.sems.allocated().values()))
    nc.all_engine_barrier()


tile.TileContext._drain_and_barrier = _patched_drain_and_barrier
# ---------------------------------------------------------------------------

AF = mybir.ActivationFunctionType
ALU = mybir.AluOpType
F32 = mybir.dt.float32
BF16 = mybir.dt.bfloat16

C = 96
KK = 9

MAX_WAITS_PER_INST = 1


def _split_multi_waits(nc):
    """This walrus build encodes at most one sem wait per instruction. Hoist
    extra waits onto NoOp carriers inserted just before, on the same engine."""
    f = nc.m.functions[0]
    for blk in f.blocks:
        insts = blk.instructions
        new = []
        changed = False
        ctr = 0
        for inst in insts:
            si = inst.sync_info
            waits = list(si.on_wait) if (si and si.on_wait) else []
            if len(waits) > MAX_WAITS_PER_INST:
                changed = True
                while len(waits) > MAX_WAITS_PER_INST:
                    chunk = waits[:MAX_WAITS_PER_INST]
                    waits = waits[MAX_WAITS_PER_INST:]
                    nop = mybir.InstNoOp(
                        name=f"{inst.name}-wsplit{ctr}", engine=inst.engine,
                        ins=[], outs=[],
                        sync_info=mybir.SyncInfo(on_wait=chunk, on_update=[]))
                    try:
                        nc.register_instruction(nop, overwrite=True)
                    except Exception:
                        pass
                    new.append(nop)
                    ctr += 1
                si.on_wait = waits
            new.append(inst)
        if changed:
            blk.instructions = new


def sub_ap(t_ap, row0, col0, nrow, ncol, rstep, cstep):
    """Strided 3D view [C, nrow, ncol] of a padded [C, PH, PW] SBUF tile."""
    base = t_ap[:, row0, col0]
    pstep = t_ap.ap[0][0]
    row_elems = t_ap.ap[-2][0]
    return bass.AP(
        tensor=base.tensor,
        offset=base.offset,
        ap=[[pstep, C], [row_elems * rstep, nrow], [cstep, ncol]],
    )


def flat_ap(t_ap, start, n):
    """1D slice [C, n] at elem offset `start` of any [C, ...] SBUF tile."""
    pstep = t_ap.ap[0][0]
    return bass.AP(tensor=t_ap.tensor, offset=t_ap.offset + start,
                   ap=[[pstep, C], [1, n]])


def build_kernel(nc, H, W, sc1, sc2, res_coef, nc1v, nc2v):
    HW = H * W
    total = float(C * HW)
    Ho = (H - 3) // 3 + 1
    Wo = (W - 3) // 3 + 1
    L = Ho * Wo
    PH, PW = H + 2, W + 4      # interior origin: row 1, col 2
    NT = H // 4                # 4-row blocks
    assert H % 4 == 0 and W % 4 == 0
    eps = 1e-5

    CW = 1024                  # phase-A chunk width (elems per partition)
    NCH = HW // CW             # 16 chunks
    RPC = CW // W              # rows per chunk (8)

    plc = max(1, min(126 // Wo, Ho))                 # ph rows per transpose blk
    npc = plc * max(1, min(504 // (plc * Wo), (Ho + plc - 1) // plc))
    lcnt_max = plc * Wo                              # 126

    # ---------------- DRAM ----------------
    x_in = nc.dram_tensor("x", [C, H * W], F32, kind="ExternalInput").ap()
    aw1t = nc.dram_tensor("aw1t", [C, KK * C], BF16, kind="ExternalInput").ap()
    aw2t = nc.dram_tensor("aw2t", [C, KK * C], BF16, kind="ExternalInput").ap()
    aw3t = nc.dram_tensor("aw3t", [C, KK * C], BF16, kind="ExternalInput").ap()
    wc2t = nc.dram_tensor("wc2t", [C, KK * C], BF16, kind="ExternalInput").ap()
    wf1 = nc.dram_tensor("wf1", [C, C], F32, kind="ExternalInput").ap()
    wf2 = nc.dram_tensor("wf2", [C, C], BF16, kind="ExternalInput").ap()
    bfull = nc.dram_tensor("bfull", [C, 1], F32, kind="ExternalInput").ap()
    out_d = nc.dram_tensor("out", [C, H * W], F32, kind="ExternalOutput").ap()

    with tile.TileContext(nc) as tc, ExitStack() as ctx:
        dram = ctx.enter_context(tc.tile_pool(name="dram", bufs=1, space="DRAM"))
        xr_d = dram.tile([C, H * W], BF16)

        # ---------------- big SBUF tiles ----------------
        big = ctx.enter_context(tc.tile_pool(name="big", bufs=1))
        x1pad = big.tile([C, PH, PW], BF16)
        x2pad = big.tile([C, PH, PW], BF16)   # later carved: y2t = flat 16384
        valp = big.tile([C, PH, PW], BF16)    # value conv output (padded)
        cv_sb = big.tile([C, HW], BF16)       # cv; earlier: pass-3 work arena
        a_sb = big.tile([C, HW], BF16)        # attn-conv out; earlier: u0 ring

        wpool = ctx.enter_context(tc.tile_pool(name="wpool", bufs=1))
        aw1_sb = wpool.tile([C, KK, C], BF16)
        aw2_sb = wpool.tile([C, KK, C], BF16)
        aw3_sb = wpool.tile([C, KK, C], BF16)
        wc2_sb = wpool.tile([C, KK, C], BF16)
        wf1_sb = wpool.tile([C, C], F32)
        wf1s_sb = wpool.tile([C, C], BF16)
        wf2_sb = wpool.tile([C, C], BF16)
        bfull_sb = wpool.tile([C, 1], F32)
        ident = wpool.tile([128, 128], BF16)
        identf = wpool.tile([128, 128], F32)
        ones_c = wpool.tile([C, 1], F32)
        ones_row = wpool.tile([1, C], F32)
        for dst, src in [(aw1_sb, aw1t), (aw2_sb, aw2t), (aw3_sb, aw3t),
                         (wc2_sb, wc2t), (wf2_sb, wf2)]:
            d = dst[:].rearrange("c a b -> c (a b)") if len(dst.shape) == 3 else dst
            nc.sync.dma_start(out=d, in_=src)
        nc.sync.dma_start(out=wf1_sb, in_=wf1)
        nc.sync.dma_start(out=bfull_sb, in_=bfull)
        make_identity(nc, ident)
        make_identity(nc, identf)
        nc.vector.memset(ones_c, 1.0)
        nc.vector.memset(ones_row, 1.0)

        # zero pad borders once (interiors rewritten below)
        for pad in (x1pad, x2pad, valp):
            nc.vector.memset(pad[:, 0, :], 0.0)
            nc.vector.memset(pad[:, PH - 1, :], 0.0)
            nc.vector.memset(pad[:, :, 0:2], 0.0)
            nc.vector.memset(pad[:, :, W + 2:W + 4], 0.0)

        # ---------------- stats / scalar pools ----------------
        st = ctx.enter_context(tc.tile_pool(name="st", bufs=1))
        bnstats = st.tile([C, 2 * NCH, 6], F32)
        mv = st.tile([C, 2], F32)
        pm = st.tile([C, 2], F32)
        s1part = st.tile([C, NCH], F32)
        q1part = st.tile([C, NCH], F32)
        pospart = st.tile([C, NCH], F32)
        p3 = st.tile([C, 3], F32)
        sc = st.tile([1, 48], F32)      # scalar scratch lane
        cstv = st.tile([1, 24], F32)    # consts to broadcast
        cst = st.tile([C, 24], F32)     # broadcast result
        astats = st.tile([C, NT, 6], F32)
        cvstats = st.tile([C, NT, 6], F32)
        amv = st.tile([C, 2], F32)
        cvmv = st.tile([C, 2], F32)
        lnp = st.tile([C, 4], F32)
        lns = st.tile([1, 8], F32)
        lnb = st.tile([C, 8], F32)
        corr = st.tile([C, 1], F32)
        attn_sb = st.tile([C, KK, C], F32)
        attnT_sb = st.tile([C, KK, C], BF16)
        mx = st.tile([C, 1], F32)
        negmx = st.tile([C, 1], F32)
        den = st.tile([C, 1], F32)
        rden = st.tile([C, 1], F32)
        cs1 = st.tile([C, 1], F32)
        kct = st.tile([1, 8], F32)
        nc.vector.memset(kct[:, 0:1], total)
        nc.vector.memset(kct[:, 1:2], eps)
        nc.vector.memset(kct[:, 2:3], float(sc1))
        nc.vector.memset(kct[:, 3:4], float(sc2))
        nc.vector.memset(kct[:, 4:5], float(res_coef))
        K_TOTAL = kct[:, 0:1]; K_EPS = kct[:, 1:2]
        K_SC1 = kct[:, 2:3]; K_SC2 = kct[:, 3:4]; K_RES = kct[:, 4:5]

        # ---------------- PSUM pools ----------------
        psC = ctx.enter_context(tc.tile_pool(name="psC", bufs=3, space="PSUM"))
        psT = ctx.enter_context(tc.tile_pool(name="psT", bufs=2, space="PSUM"))
        psA = ctx.enter_context(tc.tile_pool(name="psA", bufs=1, space="PSUM"))
        psS = ctx.enter_context(tc.tile_pool(name="psS", bufs=1, space="PSUM"))

        # ---------------- small rings ----------------
        xt_pool = ctx.enter_context(tc.tile_pool(name="xt", bufs=2))
        ev_pool = ctx.enter_context(tc.tile_pool(name="ev", bufs=2))
        kq_pool = ctx.enter_context(tc.tile_pool(name="kq", bufs=3))
        io_pool = ctx.enter_context(tc.tile_pool(name="io", bufs=2))

        def bcast(dst_ck, src_1k, k):
            """broadcast [1,k] -> [C,k] via ones matmul."""
            ps = psS.tile([C, 32], F32, tag="small")
            nc.tensor.matmul(out=ps[:, :k], lhsT=ones_row, rhs=src_1k,
                             start=True, stop=True)
            nc.vector.tensor_copy(dst_ck, ps[:, :k])

        xr_flat = xr_d[:]
        x2flat = flat_ap(x2pad[:], 0, HW)   # y2t arena (x2pad reused)

        # =========================================================
        # Phase A1: single stats pass over x
        #   bn_stats(x) -> per-channel mean/ex2;  u0 = relu(x) (Act, accum S1)
        #   Square(u0) (Act, accum Q1);  is_gt (DVE, accum POS)
        # =========================================================
        for c in range(NCH):
            xt = xt_pool.tile([C, CW], F32, tag="xt")
            nc.sync.dma_start(out=xt, in_=x_in[:, c * CW:(c + 1) * CW])
            xtv = xt.rearrange("c (a b) -> c a b", a=2)
            for j in range(2):
                nc.vector.bn_stats(out=bnstats[:, 2 * c + j, :], in_=xtv[:, j, :])
            u0 = flat_ap(a_sb[:], (c % 4) * CW, CW)
            nc.scalar.activation(out=u0, in_=xt, func=AF.Relu,
                                 accum_out=s1part[:, c:c + 1])
            sq = flat_ap(a_sb[:], (4 + (c % 4)) * CW, CW)
            nc.scalar.activation(out=sq, in_=u0, func=AF.Square,
                                 accum_out=q1part[:, c:c + 1])
            gt = flat_ap(a_sb[:], (8 + (c % 4)) * CW, CW)
            nc.vector.tensor_scalar(out=gt, in0=u0, scalar1=0.0, scalar2=None,
                                    op0=ALU.is_gt, op1=ALU.add,
                                    accum_out=pospart[:, c:c + 1])

        # =========================================================
        # Soup: aggregate + threshold-correct + branch consts
        # =========================================================
        nc.vector.bn_aggr(out=mv, in_=bnstats)
        # pm = [mean_c, ex2_c]
        nc.vector.tensor_tensor(out=pm[:, 0:1], in0=mv[:, 0:1], in1=mv[:, 0:1],
                                op=ALU.mult)
        nc.vector.tensor_tensor(out=pm[:, 1:2], in0=mv[:, 1:2], in1=pm[:, 0:1],
                                op=ALU.add)
        nc.vector.tensor_copy(pm[:, 0:1], mv[:, 0:1])
        # per-channel partial reductions for S1/Q1/POS at threshold 0
        nc.vector.reduce_sum(out=p3[:, 0:1], in_=s1part, axis=mybir.AxisListType.X)
        nc.vector.reduce_sum(out=p3[:, 1:2], in_=pospart, axis=mybir.AxisListType.X)
        nc.vector.reduce_sum(out=p3[:, 2:3], in_=q1part, axis=mybir.AxisListType.X)
        psm = psS.tile([C, 32], F32, tag="small")
        nc.tensor.matmul(out=psm[:1, 0:2], lhsT=ones_c, rhs=pm, start=True, stop=True)
        ps3 = psS.tile([C, 32], F32, tag="small")
        nc.tensor.matmul(out=ps3[:1, 0:3], lhsT=ones_c, rhs=p3, start=True, stop=True)
        Smean = sc[:, 0:1]; Sex2 = sc[:, 1:2]
        nc.vector.tensor_copy(Smean, psm[:1, 0:1])
        nc.vector.tensor_copy(Sex2, psm[:1, 1:2])
        S10 = sc[:, 2:3]; POS = sc[:, 3:4]; Q10 = sc[:, 4:5]
        nc.vector.tensor_copy(S10, ps3[:1, 0:1])
        nc.vector.tensor_copy(POS, ps3[:1, 1:2])
        nc.vector.tensor_copy(Q10, ps3[:1, 2:3])
        m_ = sc[:, 5:6]
        nc.scalar.mul(out=m_, in_=Smean, mul=1.0 / C)
        Sx2 = sc[:, 6:7]
        nc.scalar.mul(out=Sx2, in_=Sex2, mul=float(HW))
        mm_ = sc[:, 7:8]
        nc.vector.tensor_tensor(out=mm_, in0=m_, in1=m_, op=ALU.mult)
        qtot = sc[:, 8:9]
        nc.vector.scalar_tensor_tensor(out=qtot, in0=mm_, scalar=-total, in1=Sx2,
                                       op0=ALU.mult, op1=ALU.add)
        # threshold corrections: S1 = S10 - m*POS ; Q1 = Q10 - 2m*S10 + m^2*POS
        t0 = sc[:, 9:10]; t1s = sc[:, 10:11]
        S1 = sc[:, 11:12]; Q1 = sc[:, 12:13]
        nc.vector.tensor_tensor(out=t0, in0=m_, in1=POS, op=ALU.mult)
        nc.vector.tensor_tensor(out=S1, in0=S10, in1=t0, op=ALU.subtract)
        nc.vector.tensor_tensor(out=t0, in0=m_, in1=S10, op=ALU.mult)
        nc.vector.scalar_tensor_tensor(out=t0, in0=t0, scalar=-2.0, in1=Q10,
                                       op0=ALU.mult, op1=ALU.add)
        nc.vector.tensor_tensor(out=t1s, in0=mm_, in1=POS, op=ALU.mult)
        nc.vector.tensor_tensor(out=Q1, in0=t0, in1=t1s, op=ALU.add)

        NEG = sc[:, 13:14]
        # NEG = total - POS  (activation trick needs [C,1] bias; do via lane)
        nc.scalar.mul(out=NEG, in_=POS, mul=-1.0)
        nc.vector.tensor_scalar(out=NEG, in0=NEG, scalar1=total, scalar2=None,
                                op0=ALU.add)
        rPOS = sc[:, 14:15]; rNEG = sc[:, 15:16]
        nc.vector.reciprocal(out=rPOS, in_=POS)
        nc.vector.reciprocal(out=rNEG, in_=NEG)
        avg1 = sc[:, 16:17]
        nc.vector.tensor_tensor(out=avg1, in0=S1, in1=rPOS, op=ALU.mult)
        nS1 = sc[:, 17:18]
        nc.scalar.mul(out=nS1, in_=S1, mul=-1.0)
        avg2 = sc[:, 18:19]
        nc.vector.tensor_tensor(out=avg2, in0=nS1, in1=rNEG, op=ALU.mult)
        q2 = sc[:, 19:20]
        nc.vector.tensor_tensor(out=q2, in0=qtot, in1=Q1, op=ALU.subtract)

        def ln_branch(Ssum, Qsum, avg, CNT_other, CNT_own, o_mean, o_scale, tmp0):
            ta = sc[:, tmp0:tmp0 + 1]
            tb = sc[:, tmp0 + 1:tmp0 + 2]
            nc.vector.tensor_tensor(out=ta, in0=avg, in1=CNT_other, op=ALU.mult)
            nc.vector.tensor_tensor(out=tb, in0=Ssum, in1=ta, op=ALU.add)
            nc.scalar.mul(out=o_mean, in_=tb, mul=1.0 / total)
            nc.vector.tensor_tensor(out=ta, in0=avg, in1=avg, op=ALU.mult)
            nc.vector.tensor_tensor(out=ta, in0=ta, in1=CNT_other, op=ALU.mult)
            nc.vector.tensor_tensor(out=ta, in0=Qsum, in1=ta, op=ALU.add)
            nc.scalar.mul(out=ta, in_=ta, mul=1.0 / total)
            nc.vector.tensor_tensor(out=tb, in0=o_mean, in1=o_mean, op=ALU.mult)
            nc.vector.tensor_tensor(out=ta, in0=ta, in1=tb, op=ALU.subtract)
            nc.scalar.activation(out=ta, in_=ta, func=AF.Sqrt, bias=K_EPS, scale=1.0)
            nc.vector.reciprocal(out=ta, in_=ta)
            nc.scalar.mul(out=tb, in_=CNT_own, mul=1.0 / total)
            nc.scalar.activation(out=tb, in_=tb, func=AF.Sqrt, bias=0.0, scale=1.0)
            nc.vector.tensor_tensor(out=o_scale, in0=tb, in1=ta, op=ALU.mult)

        mean1 = sc[:, 20:21]; scale1 = sc[:, 21:22]
        mean2 = sc[:, 22:23]; scale2 = sc[:, 23:24]
        ln_branch(S1, Q1, avg1, NEG, POS, mean1, scale1, 24)
        ln_branch(nS1, q2, avg2, POS, NEG, mean2, scale2, 24)

        a1 = sc[:, 26:27]; b1 = sc[:, 27:28]; c1n = sc[:, 28:29]
        a2 = sc[:, 29:30]; b2 = sc[:, 30:31]; c2n = sc[:, 31:32]
        nc.scalar.activation(out=a1, in_=scale1, func=AF.Identity, bias=K_SC1, scale=1.0)
        nc.vector.tensor_tensor(out=t0, in0=scale1, in1=mean1, op=ALU.mult)
        nc.scalar.mul(out=b1, in_=t0, mul=-1.0)
        nc.vector.tensor_tensor(out=t0, in0=avg1, in1=mean1, op=ALU.subtract)
        nc.vector.tensor_tensor(out=c1n, in0=scale1, in1=t0, op=ALU.mult)
        nc.scalar.activation(out=a2, in_=scale2, func=AF.Identity, bias=K_SC2, scale=1.0)
        nc.vector.tensor_tensor(out=t0, in0=scale2, in1=mean2, op=ALU.mult)
        nc.scalar.mul(out=b2, in_=t0, mul=-1.0)
        nc.vector.tensor_tensor(out=t0, in0=avg2, in1=mean2, op=ALU.subtract)
        nc.vector.tensor_tensor(out=c2n, in0=scale2, in1=t0, op=ALU.mult)
        p1 = sc[:, 32:33]; p2 = sc[:, 33:34]
        nc.scalar.activation(out=p1, in_=scale1, func=AF.Identity,
                             bias=K_RES, scale=0.5 * nc1v)
        nc.scalar.activation(out=p2, in_=scale2, func=AF.Identity,
                             bias=K_RES, scale=0.5 * nc2v)
        q1c = sc[:, 34:35]; q2c = sc[:, 35:36]
        nc.scalar.mul(out=t0, in_=b1, mul=0.5 * nc1v)
        nc.vector.scalar_tensor_tensor(out=q1c, in0=c2n, scalar=0.5 * nc2v, in1=t0,
                                       op0=ALU.mult, op1=ALU.add)
        nc.scalar.mul(out=t0, in_=c1n, mul=0.5 * nc1v)
        nc.vector.scalar_tensor_tensor(out=q2c, in0=b2, scalar=0.5 * nc2v, in1=t0,
                                       op0=ALU.mult, op1=ALU.add)

        # sign-form consts (s = sign(x-m) in {-1,+1}):
        #  x1' = a1*u + B1*s + G1,  B1 = (b1-c1n)/2, G1 = (b1+c1n)/2
        #  x2' = -(a2*un) + B2*s + G2, B2 = (c2n-b2)/2, G2 = (c2n+b2)/2
        #  xr  = (p1/a1)*(a1 u) - (p2/a2)*(a2 un) + B3*s + G3,
        #        B3 = (q1c-q2c)/2, G3 = (q1c+q2c)/2
        # cstv cols: 0:-m 1:a1 2:-a1*m 3:-a2 4:a2*m 5:B1 6:G1 7:B2 8:G2
        #            9:B3 10:G3 11:p1/a1 12:-p2/a2
        nc.scalar.mul(out=cstv[:, 0:1], in_=m_, mul=-1.0)
        nc.vector.tensor_copy(cstv[:, 1:2], a1)
        nc.vector.tensor_tensor(out=cstv[:, 2:3], in0=a1, in1=cstv[:, 0:1],
                                op=ALU.mult)
        nc.scalar.mul(out=cstv[:, 3:4], in_=a2, mul=-1.0)
        nc.vector.tensor_tensor(out=cstv[:, 4:5], in0=a2, in1=m_, op=ALU.mult)
        # p-form: w1 = (b1-c1n)*p + c1n ; w2 = (c2n-b2)*p + b2 ;
        #         wr = (q1c-q2c)*p + q2c   with p = (xm>0) in {0,1}
        nc.vector.tensor_tensor(out=cstv[:, 5:6], in0=b1, in1=c1n, op=ALU.subtract)
        nc.vector.tensor_copy(cstv[:, 6:7], c1n)
        nc.vector.tensor_tensor(out=cstv[:, 7:8], in0=c2n, in1=b2, op=ALU.subtract)
        nc.vector.tensor_copy(cstv[:, 8:9], b2)
        nc.vector.tensor_tensor(out=cstv[:, 9:10], in0=q1c, in1=q2c, op=ALU.subtract)
        nc.vector.tensor_copy(cstv[:, 10:11], q2c)
        nc.vector.reciprocal(out=t0, in_=a1)
        nc.vector.tensor_tensor(out=cstv[:, 11:12], in0=p1, in1=t0, op=ALU.mult)
        nc.vector.reciprocal(out=t0, in_=a2)
        nc.vector.tensor_tensor(out=t1s, in0=p2, in1=t0, op=ALU.mult)
        nc.scalar.mul(out=cstv[:, 12:13], in_=t1s, mul=-1.0)
        nc.scalar.mul(out=cstv[:, 13:14], in_=p2, mul=-1.0)
        nc.vector.tensor_tensor(out=cstv[:, 14:15], in0=p2, in1=m_, op=ALU.mult)
        bcast(cst[:, 0:15], cstv[:, 0:15], 15)
        NEGM = cst[:, 0:1]; A1C = cst[:, 1:2]; A1NM = cst[:, 2:3]
        NA2C = cst[:, 3:4]; A2M = cst[:, 4:5]
        B1C = cst[:, 5:6]; G1C = cst[:, 6:7]
        B2C = cst[:, 7:8]; G2C = cst[:, 8:9]
        B3C = cst[:, 9:10]; G3C = cst[:, 10:11]
        P1R = cst[:, 11:12]; NP2R = cst[:, 12:13]
        NP2C = cst[:, 13:14]; P2M = cst[:, 14:15]

        # =========================================================
        # Pass 3: emit x1', x2' (padded tiles) + xr (DRAM, bf16)
        # work slots carved from cv_sb: 8 slots x 2 parity x CW
        # =========================================================
        def slot(k, par):
            base = cv_sb if k < 4 else a_sb
            kk_ = k if k < 4 else k - 4
            return bass.AP(tensor=base[:].tensor,
                           offset=base[:].offset + (kk_ * 4 + par) * CW,
                           ap=[[base[:].ap[0][0], C], [1, CW]])

        for c in range(NCH):
            par = c % 4
            xt = xt_pool.tile([C, CW], F32, tag="xt")
            nc.sync.dma_start(out=xt, in_=x_in[:, c * CW:(c + 1) * CW])
            s_t = slot(0, par); au_t = slot(1, par); aun_t = slot(2, par)
            aunr_t = slot(3, par)
            w1_t = slot(4, par); w2_t = slot(5, par); wr_t = slot(6, par)
            r1_t = slot(7, par); xr1_t = slot(4, par)  # reuses w1 slot
            nc.scalar.activation(out=au_t, in_=xt, func=AF.Relu, bias=A1NM, scale=A1C)
            nc.vector.tensor_scalar(out=s_t, in0=au_t, scalar1=0.0, scalar2=None,
                                    op0=ALU.is_gt)
            nc.scalar.activation(out=aun_t, in_=xt, func=AF.Relu, bias=A2M, scale=NA2C)
            nc.scalar.activation(out=aunr_t, in_=xt, func=AF.Relu, bias=P2M, scale=NP2C)
            x1v = sub_ap(x1pad[:], 1 + c * RPC, 2, RPC, W, 1, 1)
            x2v = sub_ap(x2pad[:], 1 + c * RPC, 2, RPC, W, 1, 1)
            nc.vector.tensor_scalar(out=w1_t, in0=s_t, scalar1=B1C, scalar2=G1C,
                                    op0=ALU.mult, op1=ALU.add)
            nc.vector.tensor_tensor(out=x1v, in0=au_t, in1=w1_t, op=ALU.add)
            nc.vector.tensor_scalar(out=w2_t, in0=s_t, scalar1=B2C, scalar2=G2C,
                                    op0=ALU.mult, op1=ALU.add)
            nc.vector.tensor_tensor(out=x2v, in0=w2_t, in1=aun_t, op=ALU.subtract)
            nc.vector.tensor_scalar(out=wr_t, in0=s_t, scalar1=B3C, scalar2=G3C,
                                    op0=ALU.mult, op1=ALU.add)
            nc.scalar.mul(out=r1_t, in_=au_t, mul=P1R)
            nc.vector.tensor_tensor(out=xr1_t, in0=r1_t, in1=wr_t, op=ALU.add)
            # xr = xr1 - aun_r, into p's slot (consumed by the w-ops already)
            nc.vector.tensor_tensor(out=s_t, in0=xr1_t, in1=aunr_t,
                                    op=ALU.subtract)
            nc.sync.dma_start(out=xr_flat[:, c * CW:(c + 1) * CW], in_=s_t)

        # =========================================================
        # B1: attention logits, per kk: strided K/Q convs + batched
        # transposes + attn matmul accumulation
        # =========================================================
        NSLOT = 4  # transposes batched per psT tile before eviction
        ev_ctr = 0
        for kk in range(KK):
            r, s = divmod(kk, 3)
            attn_ps = psA.tile([C, C], F32, tag="attn")
            first = True
            # accumulated list of (SBUF kq tile, col offset, lcnt)
            pend = []       # transposes in current psT not yet evicted
            done_slices = []  # (sb_tile, off, lcnt, is_q)
            tp_cur = None
            tp_used = 0

            def flush_tp():
                nonlocal tp_cur, tp_used, pend, ev_ctr
                if tp_cur is None or tp_used == 0:
                    return
                sb = kq_pool.tile([128, NSLOT * C], BF16, tag="kq")
                eng = nc.scalar if (ev_ctr % 2 == 0) else nc.vector
                ev_ctr += 1
                if eng is nc.scalar:
                    nc.scalar.activation(out=sb[:, :tp_used * C],
                                         in_=tp_cur[:, :tp_used * C], func=AF.Copy)
                else:
                    nc.vector.tensor_copy(sb[:, :tp_used * C],
                                          tp_cur[:, :tp_used * C])
                for (i, lcnt, is_q) in pend:
                    done_slices.append((sb, i * C, lcnt, is_q))
                tp_cur = None; tp_used = 0; pend = []

            def add_transpose(src_ap, lcnt, is_q):
                nonlocal tp_cur, tp_used, pend
                if tp_cur is None:
                    tp_cur = psT.tile([128, NSLOT * C], BF16, tag="tp")
                    tp_used = 0
                nc.tensor.transpose(tp_cur[:lcnt, tp_used * C:tp_used * C + C],
                                    src_ap, ident[:C, :C])
                pend.append((tp_used, lcnt, is_q))
                tp_used += 1
                if tp_used == NSLOT:
                    flush_tp()

            ph0 = 0
            while ph0 < Ho:
                this = min(npc, Ho - ph0)
                N = this * Wo
                kc_ps = psC.tile([C, 512], F32, tag="conv")
                qc_ps = psC.tile([C, 512], F32, tag="conv")
                for tap in range(KK):
                    dy, dx = divmod(tap, 3)
                    rhs1 = sub_ap(x1pad[:], 3 * ph0 + r + dy, 1 + s + dx, this, Wo, 3, 3)
                    rhs2 = sub_ap(x2pad[:], 3 * ph0 + r + dy, 1 + s + dx, this, Wo, 3, 3)
                    nc.tensor.matmul(out=kc_ps[:, :N], lhsT=aw1_sb[:, tap, :],
                                     rhs=rhs1, start=(tap == 0), stop=(tap == 8))
                    nc.tensor.matmul(out=qc_ps[:, :N], lhsT=aw2_sb[:, tap, :],
                                     rhs=rhs2, start=(tap == 0), stop=(tap == 8))
                kc = ev_pool.tile([C, 512], BF16, tag="kc")
                qc = ev_pool.tile([C, 512], BF16, tag="qc")
                nc.scalar.activation(out=kc[:, :N], in_=kc_ps[:, :N], func=AF.Copy)
                nc.vector.tensor_copy(qc[:, :N], qc_ps[:, :N])
                l0 = 0
                while l0 < N:
                    lcnt = min(lcnt_max, N - l0)
                    add_transpose(kc[:, l0:l0 + lcnt], lcnt, False)
                    add_transpose(qc[:, l0:l0 + lcnt], lcnt, True)
                    l0 += lcnt
                ph0 += this
            flush_tp()
            # attn matmuls: pair consecutive (k, q) slices in order
            ks = [d for d in done_slices if not d[3]]
            qs = [d for d in done_slices if d[3]]
            for i, ((ksb, koff, lcnt, _), (qsb, qoff, _, _)) in enumerate(zip(ks, qs)):
                nc.tensor.matmul(out=attn_ps,
                                 lhsT=qsb[:lcnt, qoff:qoff + C],
                                 rhs=ksb[:lcnt, koff:koff + C],
                                 start=(i == 0), stop=(i == len(ks) - 1))
            nc.scalar.activation(out=attn_sb[:, kk, :], in_=attn_ps, func=AF.Copy)

        # ---------------- softmax over (kk, c) ----------------
        nc.vector.reduce_max(out=mx, in_=attn_sb, axis=mybir.AxisListType.XY)
        nc.scalar.mul(out=negmx, in_=mx, mul=-1.0)
        nc.scalar.activation(out=attn_sb, in_=attn_sb, func=AF.Exp, bias=negmx,
                             scale=1.0, accum_out=den)
        nc.vector.reciprocal(out=rden, in_=den)

        # =========================================================
        # B3: Cv conv (x2pad) -> cv_sb + stats
        # =========================================================
        for yb in range(NT):
            pt = psC.tile([C, 512], F32, tag="conv")
            for tap in range(KK):
                dy, dx = divmod(tap, 3)
                rhs = sub_ap(x2pad[:], yb * 4 + dy, 1 + dx, 4, W, 1, 1)
                nc.tensor.matmul(out=pt[:, :4 * W], lhsT=wc2_sb[:, tap, :], rhs=rhs,
                                 start=(tap == 0), stop=(tap == 8))
            cv_v = flat_ap(cv_sb[:], yb * 4 * W, 4 * W)
            if yb % 2 == 0:
                nc.scalar.activation(out=cv_v, in_=pt[:, :4 * W], func=AF.Copy)
            else:
                nc.vector.tensor_copy(cv_v, pt[:, :4 * W])
            nc.vector.bn_stats(out=cvstats[:, yb, :], in_=cv_v)

        # cv LN consts
        nc.vector.bn_aggr(out=cvmv, in_=cvstats)
        nc.vector.tensor_scalar(out=lnp[:, 2:3], in0=cvmv[:, 0:1],
                                scalar1=float(HW), scalar2=None, op0=ALU.mult)
        nc.vector.tensor_tensor(out=lnp[:, 3:4], in0=cvmv[:, 0:1],
                                in1=cvmv[:, 0:1], op=ALU.mult)
        nc.vector.tensor_tensor(out=lnp[:, 3:4], in0=lnp[:, 3:4],
                                in1=cvmv[:, 1:2], op=ALU.add)
        nc.vector.tensor_scalar(out=lnp[:, 3:4], in0=lnp[:, 3:4],
                                scalar1=float(HW), scalar2=None, op0=ALU.mult)
        psc2 = psS.tile([C, 32], F32, tag="small")
        nc.tensor.matmul(out=psc2[:1, 0:2], lhsT=ones_c, rhs=lnp[:, 2:4],
                         start=True, stop=True)
        sCv = lns[:, 2:3]; qCv = lns[:, 3:4]
        nc.vector.tensor_copy(sCv, psc2[:1, 0:1])
        nc.vector.tensor_copy(qCv, psc2[:1, 1:2])

        def ln_const(ssum, qsum, o_mean, o_rs, ta, tb):
            nc.scalar.mul(out=o_mean, in_=ssum, mul=1.0 / total)
            nc.scalar.mul(out=ta, in_=qsum, mul=1.0 / total)
            nc.vector.tensor_tensor(out=tb, in0=o_mean, in1=o_mean, op=ALU.mult)
            nc.vector.tensor_tensor(out=ta, in0=ta, in1=tb, op=ALU.subtract)
            nc.scalar.activation(out=ta, in_=ta, func=AF.Sqrt, bias=K_EPS, scale=1.0)
            nc.vector.reciprocal(out=o_rs, in_=ta)

        tmpa = sc[:, 40:41]; tmpb = sc[:, 41:42]
        mCv = lns[:, 6:7]; rsCv = lns[:, 7:8]
        ln_const(sCv, qCv, mCv, rsCv, tmpa, tmpb)
        nc.vector.tensor_copy(lns[:, 0:1], mCv)
        nc.vector.tensor_copy(lns[:, 1:2], rsCv)
        bcast(lnb[:, 0:2], lns[:, 0:2], 2)
        MCV = lnb[:, 0:1]; RSCV = lnb[:, 1:2]

        # =========================================================
        # B4: value conv (x1pad) -> valp (padded, SBUF)
        # =========================================================
        for yb in range(NT):
            pt = psC.tile([C, 512], F32, tag="conv")
            for tap in range(KK):
                dy, dx = divmod(tap, 3)
                rhs = sub_ap(x1pad[:], yb * 4 + dy, 1 + dx, 4, W, 1, 1)
                nc.tensor.matmul(out=pt[:, :4 * W], lhsT=aw3_sb[:, tap, :], rhs=rhs,
                                 start=(tap == 0), stop=(tap == 8))
            vv = sub_ap(valp[:], 1 + yb * 4, 2, 4, W, 1, 1)
            if yb % 2 == 0:
                nc.vector.tensor_copy(vv, pt[:, :4 * W].rearrange(
                    "c (a b) -> c a b", a=4))
            else:
                nc.scalar.activation(out=vv, in_=pt[:, :4 * W].rearrange(
                    "c (a b) -> c a b", a=4), func=AF.Copy)

        # =========================================================
        # y2t = ((cv - mCv) * rsCv) * x1  -> x2pad arena (bf16)
        # =========================================================
        for c in range(NCH):
            cv_v = flat_ap(cv_sb[:], c * CW, CW)
            y2a = bass.AP(tensor=a_sb[:].tensor,
                          offset=a_sb[:].offset + (c % 4) * CW,
                          ap=[[a_sb[:].ap[0][0], C], [W, RPC], [1, W]])  # scratch
            nc.vector.tensor_scalar(out=y2a,
                                    in0=cv_v.rearrange("c (a b) -> c a b", a=RPC),
                                    scalar1=MCV, scalar2=RSCV,
                                    op0=ALU.subtract, op1=ALU.mult)
            x1v = sub_ap(x1pad[:], 1 + c * RPC, 2, RPC, W, 1, 1)
            y2v = bass.AP(tensor=x2pad[:].tensor,
                          offset=x2pad[:].offset + c * CW,
                          ap=[[x2pad[:].ap[0][0], C], [W, RPC], [1, W]])
            nc.vector.tensor_tensor(out=y2v, in0=y2a, in1=x1v, op=ALU.mult)

        # =========================================================
        # B5: w_attn transposes (f32 -> bf16 attnT)
        # =========================================================
        for kk in range(KK):
            tp = psA.tile([128, C], F32, tag="tpf")
            nc.tensor.transpose(tp[:C, :], attn_sb[:, kk, :], identf[:C, :C])
            nc.scalar.activation(out=attnT_sb[:, kk, :], in_=tp[:C, :], func=AF.Copy)

        # prefetch xr (DRAM) into cv_sb arena for phase D
        for c in range(4):
            nc.sync.dma_start(out=flat_ap(cv_sb[:], c * (HW // 4), HW // 4),
                              in_=xr_flat[:, c * (HW // 4):(c + 1) * (HW // 4)])

        # =========================================================
        # B6: A conv (attnT @ valp) -> a_sb + stats (rden folded at evict)
        # =========================================================
        for yb in range(NT):
            pt = psC.tile([C, 512], F32, tag="conv")
            for tap in range(KK):
                dy, dx = divmod(tap, 3)
                rhs = sub_ap(valp[:], yb * 4 + dy, 1 + dx, 4, W, 1, 1)
                nc.tensor.matmul(out=pt[:, :4 * W], lhsT=attnT_sb[:, tap, :],
                                 rhs=rhs, start=(tap == 0), stop=(tap == 8))
            a_v = flat_ap(a_sb[:], yb * 4 * W, 4 * W)
            nc.scalar.activation(out=a_v, in_=pt[:, :4 * W], func=AF.Copy,
                                 scale=rden)
            nc.vector.bn_stats(out=astats[:, yb, :], in_=a_v)

        # ---------------- A LN consts -> wf1s, corr ----------------
        nc.vector.bn_aggr(out=amv, in_=astats)
        nc.vector.tensor_scalar(out=lnp[:, 0:1], in0=amv[:, 0:1],
                                scalar1=float(HW), scalar2=None, op0=ALU.mult)
        nc.vector.tensor_tensor(out=lnp[:, 1:2], in0=amv[:, 0:1],
                                in1=amv[:, 0:1], op=ALU.mult)
        nc.vector.tensor_tensor(out=lnp[:, 1:2], in0=lnp[:, 1:2],
                                in1=amv[:, 1:2], op=ALU.add)
        nc.vector.tensor_scalar(out=lnp[:, 1:2], in0=lnp[:, 1:2],
                                scalar1=float(HW), scalar2=None, op0=ALU.mult)
        ps4 = psS.tile([C, 32], F32, tag="small")
        nc.tensor.matmul(out=ps4[:1, 0:2], lhsT=ones_c, rhs=lnp[:, 0:2],
                         start=True, stop=True)
        sA = lns[:, 4:5]; qA = lns[:, 5:6]
        nc.vector.tensor_copy(sA, ps4[:1, 0:1])
        nc.vector.tensor_copy(qA, ps4[:1, 1:2])
        mA = lns[:, 6:7]; rsA = lns[:, 7:8]
        ln_const(sA, qA, mA, rsA, tmpa, tmpb)
        nc.vector.tensor_copy(lns[:, 2:3], rsA)
        nc.vector.tensor_copy(lns[:, 3:4], mA)
        bcast(lnb[:, 2:4], lns[:, 2:4], 2)
        RSA = lnb[:, 2:3]; MA_ = lnb[:, 3:4]
        # wf1s = wf1 * rsA (bf16)
        nc.vector.tensor_scalar_mul(out=wf1s_sb, in0=wf1_sb, scalar1=RSA)
        # corr = bfull - rsA*mA*colsum(wf1)
        psc = psS.tile([C, 32], F32, tag="small")
        nc.tensor.matmul(out=psc[:, 0:1], lhsT=wf1_sb, rhs=ones_c,
                         start=True, stop=True)
        nc.vector.tensor_copy(cs1, psc[:, 0:1])
        nc.vector.tensor_scalar_mul(out=cs1, in0=cs1, scalar1=RSA)
        nc.vector.tensor_scalar_mul(out=cs1, in0=cs1, scalar1=MA_)
        nc.vector.tensor_tensor(out=corr, in0=bfull_sb, in1=cs1, op=ALU.subtract)

        # =========================================================
        # Phase D: out = (wf1s@a + wf2@y2t) + corr + xr
        # =========================================================
        for yb in range(NT):
            a_v = flat_ap(a_sb[:], yb * 4 * W, 4 * W)
            y2_v = flat_ap(x2pad[:], yb * 4 * W, 4 * W)
            xr_v = flat_ap(cv_sb[:], yb * 4 * W, 4 * W)
            pt = psC.tile([C, 512], F32, tag="conv")
            nc.tensor.matmul(out=pt[:, :4 * W], lhsT=wf1s_sb, rhs=a_v,
                             start=True, stop=False)
            nc.tensor.matmul(out=pt[:, :4 * W], lhsT=wf2_sb, rhs=y2_v,
                             start=False, stop=True)
            ot = io_pool.tile([C, 4 * W], F32, tag="ot")
            nc.vector.scalar_tensor_tensor(out=ot, in0=pt[:, :4 * W], scalar=corr,
                                           in1=xr_v, op0=ALU.add, op1=ALU.add)
            nc.sync.dma_start(out=out_d[:, yb * 4 * W:(yb + 1) * 4 * W], in_=ot)

    _split_multi_waits(nc)
    return nc


_NC_CACHE = {}


def _get_nc(H, W, sc1, sc2, res_coef, nc1v, nc2v):
    key = (H, W, float(sc1), float(sc2), float(res_coef), float(nc1v), float(nc2v))
    if key not in _NC_CACHE:
        nc = bass.Bass("TRN2", target_bir_lowering=False, debug=False)
        build_kernel(nc, H, W, float(sc1), float(sc2), float(res_coef),
                     float(nc1v), float(nc2v))
        _NC_CACHE[key] = nc
    return _NC_CACHE[key]


def _prep_w(w, scale=1.0):
    return np.ascontiguousarray(
        (np.asarray(w, np.float32).transpose(1, 2, 3, 0).reshape(C, 9 * C) * scale)
    ).astype(ml_dtypes.bfloat16)


def kernel(x, w_conv2, aw1, aw2, aw3, w_full, b_full, sc1, sc2, res_coef, nc1, nc2):
    x = np.asarray(x, np.float32)
    B, Cc, H, W = x.shape
    assert Cc == C
    nc = _get_nc(H, W, sc1, sc2, res_coef, nc1, nc2)

    inv_s = 1.0 / np.sqrt(C * 9.0)
    w_full = np.asarray(w_full, np.float32)
    shared = {
        "aw1t": _prep_w(aw1, inv_s),
        "aw2t": _prep_w(aw2),
        "aw3t": _prep_w(aw3),
        "wc2t": _prep_w(w_conv2),
        "wf1": np.ascontiguousarray(w_full[:, :C, 0, 0].T).astype(np.float32),
        "wf2": np.ascontiguousarray(w_full[:, C:, 0, 0].T).astype(ml_dtypes.bfloat16),
        "bfull": np.asarray(b_full, np.float32).reshape(C, 1),
    }
    in_maps = [{"x": np.ascontiguousarray(x[b]).reshape(C, H * W), **shared}
               for b in range(B)]
    res = run_bass_kernel_spmd(nc, in_maps, core_ids=list(range(B)))
    return np.stack([res.results[b]["out"].reshape(C, H, W) for b in range(B)],
                    axis=0)


# revision 4
# speedup vs baseline: 1.0088x; 1.0040x over previous
"""ButterflyGatingUnit Trainium2 kernel, v2.

Contract: kernel(**inputs) takes FULL inputs (x: [8, 96, 128, 128] + weights/
scalars), returns FULL output [8, 96, 128, 128] f32. Data-parallel over batch,
one example per NeuronCore, one SPMD Bass program.

v2 structure (vs baseline):
 - Phase A: ONE stats pass over x (stats of relu(x) at threshold 0 +
   analytic correction to threshold m) instead of two; sign-form emit pass
   split across Act/DVE/Pool engines, all bf16.
 - val/cv/a stay in SBUF (no DRAM round trips); y2t precomputed into x2pad's
   space; xr round-trips DRAM in bf16.
 - B1 transposes batched 5-per-PSUM-tile before eviction; evictions
   alternate Act/DVE.
"""
import numpy as np
import ml_dtypes
import concourse.bass as bass
import concourse.tile as tile
import concourse.mybir as mybir
from concourse.masks import make_identity
from concourse.bass_utils import run_bass_kernel_spmd
from contextlib import ExitStack

# ---------------------------------------------------------------------------
# Patch TileContext._drain_and_barrier: this walrus build rejects Drain
# instructions carrying more than one sem wait. Split the final global-clock
# wait set across a chain of Drain instructions on SP, one wait each.
from concourse.vector_clock import ScopedClock

MAX_WAITS_PER_DRAIN = 1


def _patched_drain_and_barrier(self, tick_clock, wait_clock):
    nc = self.nc
    drain_inst = nc.sync.drain()
    wait_clock.add_sem_waits(
        drain_inst.ins, ScopedClock({None: tick_clock.global_clock})
    )
    inst = drain_inst.ins
    si = inst.sync_info
    waits = list(si.on_wait) if (si and si.on_wait) else []
    if len(waits) > MAX_WAITS_PER_DRAIN:
        si.on_wait = waits[:MAX_WAITS_PER_DRAIN]
        rest = waits[MAX_WAITS_PER_DRAIN:]
        while rest:
            extra = nc.sync.drain()
            extra.ins.sync_info = mybir.SyncInfo(
                on_wait=rest[:MAX_WAITS_PER_DRAIN], on_update=[]
            )
            rest = rest[MAX_WAITS_PER_DRAIN:]

    nc.all_engine_barrier()
    assert self.sems is not None
    popped = nc._tile_sem_poison_stack.pop()
    assert popped is self._sem_poison
    nc.clear_and_free_semaphores(list(self.sems.allocated().values()))
    nc.all_engine_barrier()


tile.TileContext._drain_and_barrier = _patched_drain_and_barrier
# ---------------------------------------------------------------------------

AF = mybir.ActivationFunctionType
ALU = mybir.AluOpType
F32 = mybir.dt.float32
BF16 = mybir.dt.bfloat16

C = 96
KK = 9

MAX_WAITS_PER_INST = 1


def _split_multi_waits(nc):
    """This walrus build encodes at most one sem wait per instruction. Hoist
    extra waits onto NoOp carriers inserted just before, on the same engine."""
    f = nc.m.functions[0]
    for blk in f.blocks:
        insts = blk.instructions
        new = []
        changed = False
        ctr = 0
        for inst in insts:
            si = inst.sync_info
            waits = list(si.on_wait) if (si and si.on_wait) else []
            if len(waits) > MAX_WAITS_PER_INST:
                changed = True
                while len(waits) > MAX_WAITS_PER_INST:
                    chunk = waits[:MAX_WAITS_PER_INST]
                    waits = waits[MAX_WAITS_PER_INST:]
                    nop = mybir.InstNoOp(
                        name=f"{inst.name}-wsplit{ctr}", engine=inst.engine,
                        ins=[], outs=[],
                        sync_info=mybir.SyncInfo(on_wait=chunk, on_update=[]))
                    try:
                        nc.register_instruction(nop, overwrite=True)
                    except Exception:
                        pass
                    new.append(nop)
                    ctr += 1
                si.on_wait = waits
            new.append(inst)
        if changed:
            blk.instructions = new


def sub_ap(t_ap, row0, col0, nrow, ncol, rstep, cstep):
    """Strided 3D view [C, nrow, ncol] of a padded [C, PH, PW] SBUF tile."""
    base = t_ap[:, row0, col0]
    pstep = t_ap.ap[0][0]
    row_elems = t_ap.ap[-2][0]
    return bass.AP(
        tensor=base.tensor,
        offset=base.offset,
        ap=[[pstep, C], [row_elems * rstep, nrow], [cstep, ncol]],
    )


def flat_ap(t_ap, start, n):
    """1D slice [C, n] at elem offset `start` of any [C, ...] SBUF tile."""
    pstep = t_ap.ap[0][0]
    return bass.AP(tensor=t_ap.tensor, offset=t_ap.offset + start,
                   ap=[[pstep, C], [1, n]])


def build_kernel(nc, H, W, sc1, sc2, res_coef, nc1v, nc2v):
    HW = H * W
    total = float(C * HW)
    Ho = (H - 3) // 3 + 1
    Wo = (W - 3) // 3 + 1
    L = Ho * Wo
    PH, PW = H + 2, W + 4      # interior origin: row 1, col 2
    NT = H // 4                # 4-row blocks
    assert H % 4 == 0 and W % 4 == 0
    eps = 1e-5

    CW = 1024                  # phase-A chunk width (elems per partition)
    NCH = HW // CW             # 16 chunks
    RPC = CW // W              # rows per chunk (8)

    plc = max(1, min(126 // Wo, Ho))                 # ph rows per transpose blk
    npc = plc * max(1, min(504 // (plc * Wo), (Ho + plc - 1) // plc))
    lcnt_max = plc * Wo                              # 126

    # ---------------- DRAM ----------------
    x_in = nc.dram_tensor("x", [C, H * W], F32, kind="ExternalInput").ap()
    aw1t = nc.dram_tensor("aw1t", [C, KK * C], BF16, kind="ExternalInput").ap()
    aw2t = nc.dram_tensor("aw2t", [C, KK * C], BF16, kind="ExternalInput").ap()
    aw3t = nc.dram_tensor("aw3t", [C, KK * C], BF16, kind="ExternalInput").ap()
    wc2t = nc.dram_tensor("wc2t", [C, KK * C], BF16, kind="ExternalInput").ap()
    wf1 = nc.dram_tensor("wf1", [C, C], F32, kind="ExternalInput").ap()
    wf2 = nc.dram_tensor("wf2", [C, C], BF16, kind="ExternalInput").ap()
    bfull = nc.dram_tensor("bfull", [C, 1], F32, kind="ExternalInput").ap()
    out_d = nc.dram_tensor("out", [C, H * W], F32, kind="ExternalOutput").ap()

    with tile.TileContext(nc) as tc, ExitStack() as ctx:
        dram = ctx.enter_context(tc.tile_pool(name="dram", bufs=1, space="DRAM"))
        xr_d = dram.tile([C, H * W], BF16)

        # ---------------- big SBUF tiles ----------------
        big = ctx.enter_context(tc.tile_pool(name="big", bufs=1))
        x1pad = big.tile([C, PH, PW], BF16)
        x2pad = big.tile([C, PH, PW], BF16)   # later carved: y2t = flat 16384
        valp = big.tile([C, PH, PW], BF16)    # value conv output (padded)
        cv_sb = big.tile([C, HW], BF16)       # cv; earlier: pass-3 work arena
        a_sb = big.tile([C, HW], BF16)        # attn-conv out; earlier: u0 ring

        wpool = ctx.enter_context(tc.tile_pool(name="wpool", bufs=1))
        aw1_sb = wpool.tile([C, KK, C], BF16)
        aw2_sb = wpool.tile([C, KK, C], BF16)
        aw3_sb = wpool.tile([C, KK, C], BF16)
        wc2_sb = wpool.tile([C, KK, C], BF16)
        wf1_sb = wpool.tile([C, C], F32)
        wf1s_sb = wpool.tile([C, C], BF16)
        wf2_sb = wpool.tile([C, C], BF16)
        bfull_sb = wpool.tile([C, 1], F32)
        ident = wpool.tile([128, 128], BF16)
        identf = wpool.tile([128, 128], F32)
        ones_c = wpool.tile([C, 1], F32)
        ones_row = wpool.tile([1, C], F32)
        for dst, src in [(aw1_sb, aw1t), (aw2_sb, aw2t), (aw3_sb, aw3t),
                         (wc2_sb, wc2t), (wf2_sb, wf2)]:
            d = dst[:].rearrange("c a b -> c (a b)") if len(dst.shape) == 3 else dst
            nc.sync.dma_start(out=d, in_=src)
        nc.sync.dma_start(out=wf1_sb, in_=wf1)
        nc.sync.dma_start(out=bfull_sb, in_=bfull)
        make_identity(nc, ident)
        make_identity(nc, identf)
        nc.vector.memset(ones_c, 1.0)
        nc.vector.memset(ones_row, 1.0)

        # zero pad borders once (interiors rewritten below)
        for pad in (x1pad, x2pad, valp):
            nc.vector.memset(pad[:, 0, :], 0.0)
            nc.vector.memset(pad[:, PH - 1, :], 0.0)
            nc.vector.memset(pad[:, :, 0:2], 0.0)
            nc.vector.memset(pad[:, :, W + 2:W + 4], 0.0)

        # ---------------- stats / scalar pools ----------------
        st = ctx.enter_context(tc.tile_pool(name="st", bufs=1))
        bnstats = st.tile([C, 2 * NCH, 6], F32)
        mv = st.tile([C, 2], F32)
        pm = st.tile([C, 2], F32)
        s1part = st.tile([C, NCH], F32)
        q1part = st.tile([C, NCH], F32)
        pospart = st.tile([C, NCH], F32)
        p3 = st.tile([C, 3], F32)
        sc = st.tile([1, 48], F32)      # scalar scratch lane
        cstv = st.tile([1, 24], F32)    # consts to broadcast
        cst = st.tile([C, 24], F32)     # broadcast result
        astats = st.tile([C, NT, 6], F32)
        cvstats = st.tile([C, NT, 6], F32)
        amv = st.tile([C, 2], F32)
        cvmv = st.tile([C, 2], F32)
        lnp = st.tile([C, 4], F32)
        lns = st.tile([1, 8], F32)
        lnb = st.tile([C, 8], F32)
        corr = st.tile([C, 1], F32)
        attn_sb = st.tile([C, KK, C], F32)
        attnT_sb = st.tile([C, KK, C], BF16)
        mx = st.tile([C, 1], F32)
        negmx = st.tile([C, 1], F32)
        den = st.tile([C, 1], F32)
        rden = st.tile([C, 1], F32)
        cs1 = st.tile([C, 1], F32)
        kct = st.tile([1, 8], F32)
        nc.vector.memset(kct[:, 0:1], total)
        nc.vector.memset(kct[:, 1:2], eps)
        nc.vector.memset(kct[:, 2:3], float(sc1))
        nc.vector.memset(kct[:, 3:4], float(sc2))
        nc.vector.memset(kct[:, 4:5], float(res_coef))
        K_TOTAL = kct[:, 0:1]; K_EPS = kct[:, 1:2]
        K_SC1 = kct[:, 2:3]; K_SC2 = kct[:, 3:4]; K_RES = kct[:, 4:5]

        # ---------------- PSUM pools ----------------
        psC = ctx.enter_context(tc.tile_pool(name="psC", bufs=4, space="PSUM"))
        psT = ctx.enter_context(tc.tile_pool(name="psT", bufs=2, space="PSUM"))
        psA = ctx.enter_context(tc.tile_pool(name="psA", bufs=1, space="PSUM"))
        psS = ctx.enter_context(tc.tile_pool(name="psS", bufs=1, space="PSUM"))

        # ---------------- small rings ----------------
        xt_pool = ctx.enter_context(tc.tile_pool(name="xt", bufs=2))
        ev_pool = ctx.enter_context(tc.tile_pool(name="ev", bufs=2))
        kq_pool = ctx.enter_context(tc.tile_pool(name="kq", bufs=3))
        io_pool = ctx.enter_context(tc.tile_pool(name="io", bufs=2))

        def bcast(dst_ck, src_1k, k):
            """broadcast [1,k] -> [C,k] via ones matmul."""
            ps = psS.tile([C, 32], F32, tag="small")
            nc.tensor.matmul(out=ps[:, :k], lhsT=ones_row, rhs=src_1k,
                             start=True, stop=True)
            nc.vector.tensor_copy(dst_ck, ps[:, :k])

        xr_flat = xr_d[:]
        x2flat = flat_ap(x2pad[:], 0, HW)   # y2t arena (x2pad reused)

        # =========================================================
        # Phase A1: single stats pass over x
        #   bn_stats(x) -> per-channel mean/ex2;  u0 = relu(x) (Act, accum S1)
        #   Square(u0) (Act, accum Q1);  is_gt (DVE, accum POS)
        # =========================================================
        for c in range(NCH):
            xt = xt_pool.tile([C, CW], F32, tag="xt")
            nc.sync.dma_start(out=xt, in_=x_in[:, c * CW:(c + 1) * CW])
            xtv = xt.rearrange("c (a b) -> c a b", a=2)
            for j in range(2):
                nc.vector.bn_stats(out=bnstats[:, 2 * c + j, :], in_=xtv[:, j, :])
            u0 = flat_ap(a_sb[:], (c % 4) * CW, CW)
            nc.scalar.activation(out=u0, in_=xt, func=AF.Relu,
                                 accum_out=s1part[:, c:c + 1])
            sq = flat_ap(a_sb[:], (4 + (c % 4)) * CW, CW)
            nc.scalar.activation(out=sq, in_=u0, func=AF.Square,
                                 accum_out=q1part[:, c:c + 1])
            gt = flat_ap(a_sb[:], (8 + (c % 4)) * CW, CW)
            nc.vector.tensor_scalar(out=gt, in0=u0, scalar1=0.0, scalar2=None,
                                    op0=ALU.is_gt, op1=ALU.add,
                                    accum_out=pospart[:, c:c + 1])

        # =========================================================
        # Soup: aggregate + threshold-correct + branch consts
        # =========================================================
        nc.vector.bn_aggr(out=mv, in_=bnstats)
        # pm = [mean_c, ex2_c]
        nc.vector.tensor_tensor(out=pm[:, 0:1], in0=mv[:, 0:1], in1=mv[:, 0:1],
                                op=ALU.mult)
        nc.vector.tensor_tensor(out=pm[:, 1:2], in0=mv[:, 1:2], in1=pm[:, 0:1],
                                op=ALU.add)
        nc.vector.tensor_copy(pm[:, 0:1], mv[:, 0:1])
        # per-channel partial reductions for S1/Q1/POS at threshold 0
        nc.vector.reduce_sum(out=p3[:, 0:1], in_=s1part, axis=mybir.AxisListType.X)
        nc.vector.reduce_sum(out=p3[:, 1:2], in_=pospart, axis=mybir.AxisListType.X)
        nc.vector.reduce_sum(out=p3[:, 2:3], in_=q1part, axis=mybir.AxisListType.X)
        psm = psS.tile([C, 32], F32, tag="small")
        nc.tensor.matmul(out=psm[:1, 0:2], lhsT=ones_c, rhs=pm, start=True, stop=True)
        ps3 = psS.tile([C, 32], F32, tag="small")
        nc.tensor.matmul(out=ps3[:1, 0:3], lhsT=ones_c, rhs=p3, start=True, stop=True)
        Smean = sc[:, 0:1]; Sex2 = sc[:, 1:2]
        nc.vector.tensor_copy(Smean, psm[:1, 0:1])
        nc.vector.tensor_copy(Sex2, psm[:1, 1:2])
        S10 = sc[:, 2:3]; POS = sc[:, 3:4]; Q10 = sc[:, 4:5]
        nc.vector.tensor_copy(S10, ps3[:1, 0:1])
        nc.vector.tensor_copy(POS, ps3[:1, 1:2])
        nc.vector.tensor_copy(Q10, ps3[:1, 2:3])
        m_ = sc[:, 5:6]
        nc.scalar.mul(out=m_, in_=Smean, mul=1.0 / C)
        Sx2 = sc[:, 6:7]
        nc.scalar.mul(out=Sx2, in_=Sex2, mul=float(HW))
        mm_ = sc[:, 7:8]
        nc.vector.tensor_tensor(out=mm_, in0=m_, in1=m_, op=ALU.mult)
        qtot = sc[:, 8:9]
        nc.vector.scalar_tensor_tensor(out=qtot, in0=mm_, scalar=-total, in1=Sx2,
                                       op0=ALU.mult, op1=ALU.add)
        # threshold corrections: S1 = S10 - m*POS ; Q1 = Q10 - 2m*S10 + m^2*POS
        t0 = sc[:, 9:10]; t1s = sc[:, 10:11]
        S1 = sc[:, 11:12]; Q1 = sc[:, 12:13]
        nc.vector.tensor_tensor(out=t0, in0=m_, in1=POS, op=ALU.mult)
        nc.vector.tensor_tensor(out=S1, in0=S10, in1=t0, op=ALU.subtract)
        nc.vector.tensor_tensor(out=t0, in0=m_, in1=S10, op=ALU.mult)
        nc.vector.scalar_tensor_tensor(out=t0, in0=t0, scalar=-2.0, in1=Q10,
                                       op0=ALU.mult, op1=ALU.add)
        nc.vector.tensor_tensor(out=t1s, in0=mm_, in1=POS, op=ALU.mult)
        nc.vector.tensor_tensor(out=Q1, in0=t0, in1=t1s, op=ALU.add)

        NEG = sc[:, 13:14]
        # NEG = total - POS  (activation trick needs [C,1] bias; do via lane)
        nc.scalar.mul(out=NEG, in_=POS, mul=-1.0)
        nc.vector.tensor_scalar(out=NEG, in0=NEG, scalar1=total, scalar2=None,
                                op0=ALU.add)
        rPOS = sc[:, 14:15]; rNEG = sc[:, 15:16]
        nc.vector.reciprocal(out=rPOS, in_=POS)
        nc.vector.reciprocal(out=rNEG, in_=NEG)
        avg1 = sc[:, 16:17]
        nc.vector.tensor_tensor(out=avg1, in0=S1, in1=rPOS, op=ALU.mult)
        nS1 = sc[:, 17:18]
        nc.scalar.mul(out=nS1, in_=S1, mul=-1.0)
        avg2 = sc[:, 18:19]
        nc.vector.tensor_tensor(out=avg2, in0=nS1, in1=rNEG, op=ALU.mult)
        q2 = sc[:, 19:20]
        nc.vector.tensor_tensor(out=q2, in0=qtot, in1=Q1, op=ALU.subtract)

        def ln_branch(Ssum, Qsum, avg, CNT_other, CNT_own, o_mean, o_scale, tmp0):
            ta = sc[:, tmp0:tmp0 + 1]
            tb = sc[:, tmp0 + 1:tmp0 + 2]
            nc.vector.tensor_tensor(out=ta, in0=avg, in1=CNT_other, op=ALU.mult)
            nc.vector.tensor_tensor(out=tb, in0=Ssum, in1=ta, op=ALU.add)
            nc.scalar.mul(out=o_mean, in_=tb, mul=1.0 / total)
            nc.vector.tensor_tensor(out=ta, in0=avg, in1=avg, op=ALU.mult)
            nc.vector.tensor_tensor(out=ta, in0=ta, in1=CNT_other, op=ALU.mult)
            nc.vector.tensor_tensor(out=ta, in0=Qsum, in1=ta, op=ALU.add)
            nc.scalar.mul(out=ta, in_=ta, mul=1.0 / total)
            nc.vector.tensor_tensor(out=tb, in0=o_mean, in1=o_mean, op=ALU.mult)
            nc.vector.tensor_tensor(out=ta, in0=ta, in1=tb, op=ALU.subtract)
            nc.scalar.activation(out=ta, in_=ta, func=AF.Sqrt, bias=K_EPS, scale=1.0)
            nc.vector.reciprocal(out=ta, in_=ta)
            nc.scalar.mul(out=tb, in_=CNT_own, mul=1.0 / total)
            nc.scalar.activation(out=tb, in_=tb, func=AF.Sqrt, bias=0.0, scale=1.0)
            nc.vector.tensor_tensor(out=o_scale, in0=tb, in1=ta, op=ALU.mult)

        mean1 = sc[:, 20:21]; scale1 = sc[:, 21:22]
        mean2 = sc[:, 22:23]; scale2 = sc[:, 23:24]
        ln_branch(S1, Q1, avg1, NEG, POS, mean1, scale1, 24)
        ln_branch(nS1, q2, avg2, POS, NEG, mean2, scale2, 24)

        a1 = sc[:, 26:27]; b1 = sc[:, 27:28]; c1n = sc[:, 28:29]
        a2 = sc[:, 29:30]; b2 = sc[:, 30:31]; c2n = sc[:, 31:32]
        nc.scalar.activation(out=a1, in_=scale1, func=AF.Identity, bias=K_SC1, scale=1.0)
        nc.vector.tensor_tensor(out=t0, in0=scale1, in1=mean1, op=ALU.mult)
        nc.scalar.mul(out=b1, in_=t0, mul=-1.0)
        nc.vector.tensor_tensor(out=t0, in0=avg1, in1=mean1, op=ALU.subtract)
        nc.vector.tensor_tensor(out=c1n, in0=scale1, in1=t0, op=ALU.mult)
        nc.scalar.activation(out=a2, in_=scale2, func=AF.Identity, bias=K_SC2, scale=1.0)
        nc.vector.tensor_tensor(out=t0, in0=scale2, in1=mean2, op=ALU.mult)
        nc.scalar.mul(out=b2, in_=t0, mul=-1.0)
        nc.vector.tensor_tensor(out=t0, in0=avg2, in1=mean2, op=ALU.subtract)
        nc.vector.tensor_tensor(out=c2n, in0=scale2, in1=t0, op=ALU.mult)
        p1 = sc[:, 32:33]; p2 = sc[:, 33:34]
        nc.scalar.activation(out=p1, in_=scale1, func=AF.Identity,
                             bias=K_RES, scale=0.5 * nc1v)
        nc.scalar.activation(out=p2, in_=scale2, func=AF.Identity,
                             bias=K_RES, scale=0.5 * nc2v)
        q1c = sc[:, 34:35]; q2c = sc[:, 35:36]
        nc.scalar.mul(out=t0, in_=b1, mul=0.5 * nc1v)
        nc.vector.scalar_tensor_tensor(out=q1c, in0=c2n, scalar=0.5 * nc2v, in1=t0,
                                       op0=ALU.mult, op1=ALU.add)
        nc.scalar.mul(out=t0, in_=c1n, mul=0.5 * nc1v)
        nc.vector.scalar_tensor_tensor(out=q2c, in0=b2, scalar=0.5 * nc2v, in1=t0,
                                       op0=ALU.mult, op1=ALU.add)

        # sign-form consts (s = sign(x-m) in {-1,+1}):
        #  x1' = a1*u + B1*s + G1,  B1 = (b1-c1n)/2, G1 = (b1+c1n)/2
        #  x2' = -(a2*un) + B2*s + G2, B2 = (c2n-b2)/2, G2 = (c2n+b2)/2
        #  xr  = (p1/a1)*(a1 u) - (p2/a2)*(a2 un) + B3*s + G3,
        #        B3 = (q1c-q2c)/2, G3 = (q1c+q2c)/2
        # cstv cols: 0:-m 1:a1 2:-a1*m 3:-a2 4:a2*m 5:B1 6:G1 7:B2 8:G2
        #            9:B3 10:G3 11:p1/a1 12:-p2/a2
        nc.scalar.mul(out=cstv[:, 0:1], in_=m_, mul=-1.0)
        nc.vector.tensor_copy(cstv[:, 1:2], a1)
        nc.vector.tensor_tensor(out=cstv[:, 2:3], in0=a1, in1=cstv[:, 0:1],
                                op=ALU.mult)
        nc.scalar.mul(out=cstv[:, 3:4], in_=a2, mul=-1.0)
        nc.vector.tensor_tensor(out=cstv[:, 4:5], in0=a2, in1=m_, op=ALU.mult)
        # p-form: w1 = (b1-c1n)*p + c1n ; w2 = (c2n-b2)*p + b2 ;
        #         wr = (q1c-q2c)*p + q2c   with p = (xm>0) in {0,1}
        nc.vector.tensor_tensor(out=cstv[:, 5:6], in0=b1, in1=c1n, op=ALU.subtract)
        nc.vector.tensor_copy(cstv[:, 6:7], c1n)
        nc.vector.tensor_tensor(out=cstv[:, 7:8], in0=c2n, in1=b2, op=ALU.subtract)
        nc.vector.tensor_copy(cstv[:, 8:9], b2)
        nc.vector.tensor_tensor(out=cstv[:, 9:10], in0=q1c, in1=q2c, op=ALU.subtract)
        nc.vector.tensor_copy(cstv[:, 10:11], q2c)
        nc.vector.reciprocal(out=t0, in_=a1)
        nc.vector.tensor_tensor(out=cstv[:, 11:12], in0=p1, in1=t0, op=ALU.mult)
        nc.vector.reciprocal(out=t0, in_=a2)
        nc.vector.tensor_tensor(out=t1s, in0=p2, in1=t0, op=ALU.mult)
        nc.scalar.mul(out=cstv[:, 12:13], in_=t1s, mul=-1.0)
        nc.scalar.mul(out=cstv[:, 13:14], in_=p2, mul=-1.0)
        nc.vector.tensor_tensor(out=cstv[:, 14:15], in0=p2, in1=m_, op=ALU.mult)
        bcast(cst[:, 0:15], cstv[:, 0:15], 15)
        NEGM = cst[:, 0:1]; A1C = cst[:, 1:2]; A1NM = cst[:, 2:3]
        NA2C = cst[:, 3:4]; A2M = cst[:, 4:5]
        B1C = cst[:, 5:6]; G1C = cst[:, 6:7]
        B2C = cst[:, 7:8]; G2C = cst[:, 8:9]
        B3C = cst[:, 9:10]; G3C = cst[:, 10:11]
        P1R = cst[:, 11:12]; NP2R = cst[:, 12:13]
        NP2C = cst[:, 13:14]; P2M = cst[:, 14:15]

        # =========================================================
        # Pass 3: emit x1', x2' (padded tiles) + xr (DRAM, bf16)
        # work slots carved from cv_sb: 8 slots x 2 parity x CW
        # =========================================================
        def slot(k, par):
            base = cv_sb if k < 4 else a_sb
            kk_ = k if k < 4 else k - 4
            return bass.AP(tensor=base[:].tensor,
                           offset=base[:].offset + (kk_ * 4 + par) * CW,
                           ap=[[base[:].ap[0][0], C], [1, CW]])

        for c in range(NCH):
            par = c % 4
            xt = xt_pool.tile([C, CW], F32, tag="xt")
            nc.sync.dma_start(out=xt, in_=x_in[:, c * CW:(c + 1) * CW])
            s_t = slot(0, par); au_t = slot(1, par); aun_t = slot(2, par)
            aunr_t = slot(3, par)
            w1_t = slot(4, par); w2_t = slot(5, par); wr_t = slot(6, par)
            r1_t = slot(7, par); xr1_t = slot(4, par)  # reuses w1 slot
            nc.scalar.activation(out=au_t, in_=xt, func=AF.Relu, bias=A1NM, scale=A1C)
            nc.vector.tensor_scalar(out=s_t, in0=au_t, scalar1=0.0, scalar2=None,
                                    op0=ALU.is_gt)
            nc.scalar.activation(out=aun_t, in_=xt, func=AF.Relu, bias=A2M, scale=NA2C)
            nc.scalar.activation(out=aunr_t, in_=xt, func=AF.Relu, bias=P2M, scale=NP2C)
            x1v = sub_ap(x1pad[:], 1 + c * RPC, 2, RPC, W, 1, 1)
            x2v = sub_ap(x2pad[:], 1 + c * RPC, 2, RPC, W, 1, 1)
            nc.vector.tensor_scalar(out=w1_t, in0=s_t, scalar1=B1C, scalar2=G1C,
                                    op0=ALU.mult, op1=ALU.add)
            nc.vector.tensor_tensor(out=x1v, in0=au_t, in1=w1_t, op=ALU.add)
            nc.vector.tensor_scalar(out=w2_t, in0=s_t, scalar1=B2C, scalar2=G2C,
                                    op0=ALU.mult, op1=ALU.add)
            nc.vector.tensor_tensor(out=x2v, in0=w2_t, in1=aun_t, op=ALU.subtract)
            nc.vector.tensor_scalar(out=wr_t, in0=s_t, scalar1=B3C, scalar2=G3C,
                                    op0=ALU.mult, op1=ALU.add)
            nc.scalar.mul(out=r1_t, in_=au_t, mul=P1R)
            nc.vector.tensor_tensor(out=xr1_t, in0=r1_t, in1=wr_t, op=ALU.add)
            # xr = xr1 - aun_r, into p's slot (consumed by the w-ops already)
            nc.vector.tensor_tensor(out=s_t, in0=xr1_t, in1=aunr_t,
                                    op=ALU.subtract)
            nc.sync.dma_start(out=xr_flat[:, c * CW:(c + 1) * CW], in_=s_t)

        # =========================================================
        # B1: attention logits, per kk: strided K/Q convs + batched
        # transposes + attn matmul accumulation
        # =========================================================
        NSLOT = 4  # transposes batched per psT tile before eviction
        ev_ctr = 0
        for kk in range(KK):
            r, s = divmod(kk, 3)
            attn_ps = psA.tile([C, C], F32, tag="attn")
            first = True
            # accumulated list of (SBUF kq tile, col offset, lcnt)
            pend = []       # transposes in current psT not yet evicted
            done_slices = []  # (sb_tile, off, lcnt, is_q)
            tp_cur = None
            tp_used = 0

            def flush_tp():
                nonlocal tp_cur, tp_used, pend, ev_ctr
                if tp_cur is None or tp_used == 0:
                    return
                sb = kq_pool.tile([128, NSLOT * C], BF16, tag="kq")
                eng = nc.scalar if (ev_ctr % 2 == 0) else nc.vector
                ev_ctr += 1
                if eng is nc.scalar:
                    nc.scalar.activation(out=sb[:, :tp_used * C],
                                         in_=tp_cur[:, :tp_used * C], func=AF.Copy)
                else:
                    nc.vector.tensor_copy(sb[:, :tp_used * C],
                                          tp_cur[:, :tp_used * C])
                for (i, lcnt, is_q) in pend:
                    done_slices.append((sb, i * C, lcnt, is_q))
                tp_cur = None; tp_used = 0; pend = []

            def add_transpose(src_ap, lcnt, is_q):
                nonlocal tp_cur, tp_used, pend
                if tp_cur is None:
                    tp_cur = psT.tile([128, NSLOT * C], BF16, tag="tp")
                    tp_used = 0
                nc.tensor.transpose(tp_cur[:lcnt, tp_used * C:tp_used * C + C],
                                    src_ap, ident[:C, :C])
                pend.append((tp_used, lcnt, is_q))
                tp_used += 1
                if tp_used == NSLOT:
                    flush_tp()

            ph0 = 0
            while ph0 < Ho:
                this = min(npc, Ho - ph0)
                N = this * Wo
                kc_ps = psC.tile([C, 512], F32, tag="conv")
                qc_ps = psC.tile([C, 512], F32, tag="conv")
                for tap in range(KK):
                    dy, dx = divmod(tap, 3)
                    rhs1 = sub_ap(x1pad[:], 3 * ph0 + r + dy, 1 + s + dx, this, Wo, 3, 3)
                    rhs2 = sub_ap(x2pad[:], 3 * ph0 + r + dy, 1 + s + dx, this, Wo, 3, 3)
                    nc.tensor.matmul(out=kc_ps[:, :N], lhsT=aw1_sb[:, tap, :],
                                     rhs=rhs1, start=(tap == 0), stop=(tap == 8))
                    nc.tensor.matmul(out=qc_ps[:, :N], lhsT=aw2_sb[:, tap, :],
                                     rhs=rhs2, start=(tap == 0), stop=(tap == 8))
                kc = ev_pool.tile([C, 512], BF16, tag="kc")
                qc = ev_pool.tile([C, 512], BF16, tag="qc")
                nc.scalar.activation(out=kc[:, :N], in_=kc_ps[:, :N], func=AF.Copy)
                nc.vector.tensor_copy(qc[:, :N], qc_ps[:, :N])
                l0 = 0
                while l0 < N:
                    lcnt = min(lcnt_max, N - l0)
                    add_transpose(kc[:, l0:l0 + lcnt], lcnt, False)
                    add_transpose(qc[:, l0:l0 + lcnt], lcnt, True)
                    l0 += lcnt
                ph0 += this
            flush_tp()
            # attn matmuls: pair consecutive (k, q) slices in order
            ks = [d for d in done_slices if not d[3]]
            qs = [d for d in done_slices if d[3]]
            for i, ((ksb, koff, lcnt, _), (qsb, qoff, _, _)) in enumerate(zip(ks, qs)):
                nc.tensor.matmul(out=attn_ps,
                                 lhsT=qsb[:lcnt, qoff:qoff + C],
                                 rhs=ksb[:lcnt, koff:koff + C],
                                 start=(i == 0), stop=(i == len(ks) - 1))
            nc.scalar.activation(out=attn_sb[:, kk, :], in_=attn_ps, func=AF.Copy)

        # ---------------- softmax over (kk, c) ----------------
        nc.vector.reduce_max(out=mx, in_=attn_sb, axis=mybir.AxisListType.XY)
        nc.scalar.mul(out=negmx, in_=mx, mul=-1.0)
        nc.scalar.activation(out=attn_sb, in_=attn_sb, func=AF.Exp, bias=negmx,
                             scale=1.0, accum_out=den)
        nc.vector.reciprocal(out=rden, in_=den)

        # =========================================================
        # B3: Cv conv (x2pad) -> cv_sb + stats
        # =========================================================
        for yb in range(NT):
            pt = psC.tile([C, 512], F32, tag="conv")
            for tap in range(KK):
                dy, dx = divmod(tap, 3)
                rhs = sub_ap(x2pad[:], yb * 4 + dy, 1 + dx, 4, W, 1, 1)
                nc.tensor.matmul(out=pt[:, :4 * W], lhsT=wc2_sb[:, tap, :], rhs=rhs,
                                 start=(tap == 0), stop=(tap == 8))
            cv_v = flat_ap(cv_sb[:], yb * 4 * W, 4 * W)
            if yb % 2 == 0:
                nc.scalar.activation(out=cv_v, in_=pt[:, :4 * W], func=AF.Copy)
            else:
                nc.vector.tensor_copy(cv_v, pt[:, :4 * W])
            nc.vector.bn_stats(out=cvstats[:, yb, :], in_=cv_v)

        # cv LN consts
        nc.vector.bn_aggr(out=cvmv, in_=cvstats)
        nc.vector.tensor_scalar(out=lnp[:, 2:3], in0=cvmv[:, 0:1],
                                scalar1=float(HW), scalar2=None, op0=ALU.mult)
        nc.vector.tensor_tensor(out=lnp[:, 3:4], in0=cvmv[:, 0:1],
                                in1=cvmv[:, 0:1], op=ALU.mult)
        nc.vector.tensor_tensor(out=lnp[:, 3:4], in0=lnp[:, 3:4],
                                in1=cvmv[:, 1:2], op=ALU.add)
        nc.vector.tensor_scalar(out=lnp[:, 3:4], in0=lnp[:, 3:4],
                                scalar1=float(HW), scalar2=None, op0=ALU.mult)
        psc2 = psS.tile([C, 32], F32, tag="small")
        nc.tensor.matmul(out=psc2[:1, 0:2], lhsT=ones_c, rhs=lnp[:, 2:4],
                         start=True, stop=True)
        sCv = lns[:, 2:3]; qCv = lns[:, 3:4]
        nc.vector.tensor_copy(sCv, psc2[:1, 0:1])
        nc.vector.tensor_copy(qCv, psc2[:1, 1:2])

        def ln_const(ssum, qsum, o_mean, o_rs, ta, tb):
            nc.scalar.mul(out=o_mean, in_=ssum, mul=1.0 / total)
            nc.scalar.mul(out=ta, in_=qsum, mul=1.0 / total)
            nc.vector.tensor_tensor(out=tb, in0=o_mean, in1=o_mean, op=ALU.mult)
            nc.vector.tensor_tensor(out=ta, in0=ta, in1=tb, op=ALU.subtract)
            nc.scalar.activation(out=ta, in_=ta, func=AF.Sqrt, bias=K_EPS, scale=1.0)
            nc.vector.reciprocal(out=o_rs, in_=ta)

        tmpa = sc[:, 40:41]; tmpb = sc[:, 41:42]
        mCv = lns[:, 6:7]; rsCv = lns[:, 7:8]
        ln_const(sCv, qCv, mCv, rsCv, tmpa, tmpb)
        nc.vector.tensor_copy(lns[:, 0:1], mCv)
        nc.vector.tensor_copy(lns[:, 1:2], rsCv)
        bcast(lnb[:, 0:2], lns[:, 0:2], 2)
        MCV = lnb[:, 0:1]; RSCV = lnb[:, 1:2]

        # =========================================================
        # B4: value conv (x1pad) -> valp (padded, SBUF)
        # =========================================================
        for yb in range(NT):
            pt = psC.tile([C, 512], F32, tag="conv")
            for tap in range(KK):
                dy, dx = divmod(tap, 3)
                rhs = sub_ap(x1pad[:], yb * 4 + dy, 1 + dx, 4, W, 1, 1)
                nc.tensor.matmul(out=pt[:, :4 * W], lhsT=aw3_sb[:, tap, :], rhs=rhs,
                                 start=(tap == 0), stop=(tap == 8))
            vv = sub_ap(valp[:], 1 + yb * 4, 2, 4, W, 1, 1)
            if yb % 2 == 0:
                nc.vector.tensor_copy(vv, pt[:, :4 * W].rearrange(
                    "c (a b) -> c a b", a=4))
            else:
                nc.scalar.activation(out=vv, in_=pt[:, :4 * W].rearrange(
                    "c (a b) -> c a b", a=4), func=AF.Copy)

        # =========================================================
        # y2t = ((cv - mCv) * rsCv) * x1  -> x2pad arena (bf16)
        # =========================================================
        for c in range(NCH):
            cv_v = flat_ap(cv_sb[:], c * CW, CW)
            y2a = bass.AP(tensor=a_sb[:].tensor,
                          offset=a_sb[:].offset + (c % 4) * CW,
                          ap=[[a_sb[:].ap[0][0], C], [W, RPC], [1, W]])  # scratch
            nc.vector.tensor_scalar(out=y2a,
                                    in0=cv_v.rearrange("c (a b) -> c a b", a=RPC),
                                    scalar1=MCV, scalar2=RSCV,
                                    op0=ALU.subtract, op1=ALU.mult)
            x1v = sub_ap(x1pad[:], 1 + c * RPC, 2, RPC, W, 1, 1)
            y2v = bass.AP(tensor=x2pad[:].tensor,
                          offset=x2pad[:].offset + c * CW,
                          ap=[[x2pad[:].ap[0][0], C], [W, RPC], [1, W]])
            nc.vector.tensor_tensor(out=y2v, in0=y2a, in1=x1v, op=ALU.mult)

        # =========================================================
        # B5: w_attn transposes (f32 -> bf16 attnT)
        # =========================================================
        attnb = st.tile([C, KK, C], BF16)
        nc.scalar.activation(out=attnb[:].rearrange("c a b -> c (a b)"),
                             in_=attn_sb[:].rearrange("c a b -> c (a b)"),
                             func=AF.Copy)
        for kk in range(KK):
            tp = psT.tile([128, NSLOT * C], BF16, tag="tp")
            nc.tensor.transpose(tp[:C, :C], attnb[:, kk, :], ident[:C, :C])
            nc.scalar.activation(out=attnT_sb[:, kk, :], in_=tp[:C, :C], func=AF.Copy)

        # prefetch xr (DRAM) into cv_sb arena for phase D
        for c in range(4):
            nc.sync.dma_start(out=flat_ap(cv_sb[:], c * (HW // 4), HW // 4),
                              in_=xr_flat[:, c * (HW // 4):(c + 1) * (HW // 4)])

        # =========================================================
        # B6: A conv (attnT @ valp) -> a_sb + stats (rden folded at evict)
        # =========================================================
        for yb in range(NT):
            pt = psC.tile([C, 512], F32, tag="conv")
            for tap in range(KK):
                dy, dx = divmod(tap, 3)
                rhs = sub_ap(valp[:], yb * 4 + dy, 1 + dx, 4, W, 1, 1)
                nc.tensor.matmul(out=pt[:, :4 * W], lhsT=attnT_sb[:, tap, :],
                                 rhs=rhs, start=(tap == 0), stop=(tap == 8))
            a_v = flat_ap(a_sb[:], yb * 4 * W, 4 * W)
            nc.scalar.activation(out=a_v, in_=pt[:, :4 * W], func=AF.Copy,
                                 scale=rden)
            nc.vector.bn_stats(out=astats[:, yb, :], in_=a_v)

        # ---------------- A LN consts -> wf1s, corr ----------------
        nc.vector.bn_aggr(out=amv, in_=astats)
        nc.vector.tensor_scalar(out=lnp[:, 0:1], in0=amv[:, 0:1],
                                scalar1=float(HW), scalar2=None, op0=ALU.mult)
        nc.vector.tensor_tensor(out=lnp[:, 1:2], in0=amv[:, 0:1],
                                in1=amv[:, 0:1], op=ALU.mult)
        nc.vector.tensor_tensor(out=lnp[:, 1:2], in0=lnp[:, 1:2],
                                in1=amv[:, 1:2], op=ALU.add)
        nc.vector.tensor_scalar(out=lnp[:, 1:2], in0=lnp[:, 1:2],
                                scalar1=float(HW), scalar2=None, op0=ALU.mult)
        ps4 = psS.tile([C, 32], F32, tag="small")
        nc.tensor.matmul(out=ps4[:1, 0:2], lhsT=ones_c, rhs=lnp[:, 0:2],
                         start=True, stop=True)
        sA = lns[:, 4:5]; qA = lns[:, 5:6]
        nc.vector.tensor_copy(sA, ps4[:1, 0:1])
        nc.vector.tensor_copy(qA, ps4[:1, 1:2])
        mA = lns[:, 6:7]; rsA = lns[:, 7:8]
        ln_const(sA, qA, mA, rsA, tmpa, tmpb)
        nc.vector.tensor_copy(lns[:, 2:3], rsA)
        nc.vector.tensor_copy(lns[:, 3:4], mA)
        bcast(lnb[:, 2:4], lns[:, 2:4], 2)
        RSA = lnb[:, 2:3]; MA_ = lnb[:, 3:4]
        # wf1s = wf1 * rsA (bf16)
        nc.vector.tensor_scalar_mul(out=wf1s_sb, in0=wf1_sb, scalar1=RSA)
        # corr = bfull - rsA*mA*colsum(wf1)
        psc = psS.tile([C, 32], F32, tag="small")
        nc.tensor.matmul(out=psc[:, 0:1], lhsT=wf1_sb, rhs=ones_c,
                         start=True, stop=True)
        nc.vector.tensor_copy(cs1, psc[:, 0:1])
        nc.vector.tensor_scalar_mul(out=cs1, in0=cs1, scalar1=RSA)
        nc.vector.tensor_scalar_mul(out=cs1, in0=cs1, scalar1=MA_)
        nc.vector.tensor_tensor(out=corr, in0=bfull_sb, in1=cs1, op=ALU.subtract)

        # =========================================================
        # Phase D: out = (wf1s@a + wf2@y2t) + corr + xr
        # =========================================================
        for yb in range(NT):
            a_v = flat_ap(a_sb[:], yb * 4 * W, 4 * W)
            y2_v = flat_ap(x2pad[:], yb * 4 * W, 4 * W)
            xr_v = flat_ap(cv_sb[:], yb * 4 * W, 4 * W)
            pt = psC.tile([C, 512], F32, tag="conv")
            nc.tensor.matmul(out=pt[:, :4 * W], lhsT=wf1s_sb, rhs=a_v,
                             start=True, stop=False)
            nc.tensor.matmul(out=pt[:, :4 * W], lhsT=wf2_sb, rhs=y2_v,
                             start=False, stop=True)
            ot = io_pool.tile([C, 4 * W], F32, tag="ot")
            nc.vector.scalar_tensor_tensor(out=ot, in0=pt[:, :4 * W], scalar=corr,
                                           in1=xr_v, op0=ALU.add, op1=ALU.add)
            nc.sync.dma_start(out=out_d[:, yb * 4 * W:(yb + 1) * 4 * W], in_=ot)

    _split_multi_waits(nc)
    return nc


_NC_CACHE = {}


def _get_nc(H, W, sc1, sc2, res_coef, nc1v, nc2v):
    key = (H, W, float(sc1), float(sc2), float(res_coef), float(nc1v), float(nc2v))
    if key not in _NC_CACHE:
        nc = bass.Bass("TRN2", target_bir_lowering=False, debug=False)
        build_kernel(nc, H, W, float(sc1), float(sc2), float(res_coef),
                     float(nc1v), float(nc2v))
        _NC_CACHE[key] = nc
    return _NC_CACHE[key]


def _prep_w(w, scale=1.0):
    return np.ascontiguousarray(
        (np.asarray(w, np.float32).transpose(1, 2, 3, 0).reshape(C, 9 * C) * scale)
    ).astype(ml_dtypes.bfloat16)


def kernel(x, w_conv2, aw1, aw2, aw3, w_full, b_full, sc1, sc2, res_coef, nc1, nc2):
    x = np.asarray(x, np.float32)
    B, Cc, H, W = x.shape
    assert Cc == C
    nc = _get_nc(H, W, sc1, sc2, res_coef, nc1, nc2)

    inv_s = 1.0 / np.sqrt(C * 9.0)
    w_full = np.asarray(w_full, np.float32)
    shared = {
        "aw1t": _prep_w(aw1, inv_s),
        "aw2t": _prep_w(aw2),
        "aw3t": _prep_w(aw3),
        "wc2t": _prep_w(w_conv2),
        "wf1": np.ascontiguousarray(w_full[:, :C, 0, 0].T).astype(np.float32),
        "wf2": np.ascontiguousarray(w_full[:, C:, 0, 0].T).astype(ml_dtypes.bfloat16),
        "bfull": np.asarray(b_full, np.float32).reshape(C, 1),
    }
    in_maps = [{"x": np.ascontiguousarray(x[b]).reshape(C, H * W), **shared}
               for b in range(B)]
    res = run_bass_kernel_spmd(nc, in_maps, core_ids=list(range(B)))
    return np.stack([res.results[b]["out"].reshape(C, H, W) for b in range(B)],
                    axis=0)
